# revision 14
# baseline (speedup 1.0000x reference)
"""Trainium2 Bass kernel for MiniSelfAttention (v2).

Shapes (full problem): x (4, 2048, 1024), Wq/Wk/Wv/Wo (1024, 1024), bo (1024,).
H=16 heads, D=64. out = softmax(q k^T / 8) v  projected by Wo.

Sharding across 8 cores: core c -> batch b = c//2, head-group g = c%2
(8 heads = 512 features per group). Each core computes a partial output
projection (its 512 ctx features x Wo slice); host sums the two partials
per batch and adds the bias.

v2 structure (vs v1 baseline):
  - Single PSUM region, 8 banks: 'sc' [128,1024]f32 x2 (scores, double
    buffered, 4 banks), 'cx' [128,512]f32 x3 (AV accumulators), 'fill'
    [128,512]f32 x1 (projection / v / out-proj tiles).
  - Loop order: head-pair OUTER, q-block (j) INNER. Projections for
    chunk m+1, v tiles, and out-proj row tiles are interleaved into the
    attention s-loops as PE "filler" so the PE stays dense (HAM warm)
    and the serial phase-1 head is minimized.
  - exp split between ScalarE (true exp) and VectorE (Schraudolph
    bit-trick: bf16_bits = round_i16(A*score + B), ~2-3% sawtooth error
    that cancels in softmax) to relieve the ScalarE bottleneck.
  - softmax: Z row (ones column in vaug) -> ScalarE copy to SBUF ->
    DVE reciprocal_approx_fast -> gpsimd partition_broadcast -> one
    fused tensor_mul into ctxT. (reciprocal_approx_fast must NOT read
    PSUM directly - it misreads partition-offset rows of bank-offset
    PSUM tiles.)
"""

import sys

sys.path.insert(0, "/opt/trn_rl_repo")

import numpy as np

import concourse.bacc as bacc
import concourse.mybir as mybir
from concourse import tile
from concourse.bass_utils import run_bass_kernel_spmd

F32 = mybir.dt.float32
BF16 = mybir.dt.bfloat16
I16 = mybir.dt.int16
AF = mybir.ActivationFunctionType
ALU = mybir.AluOpType

DIM = 1024
HEADS = 16
D = 64
N_CORES = 8

LOG2E = 1.4426950408889634
EXP_A = 128.0 * LOG2E / 8.0      # folds the 1/sqrt(D) score scale
EXP_B = 16248.6                  # 127*128 tuned: centers the sawtooth error
DVE_S = ()            # s-tiles whose exp runs on VectorE

DEBUG = False                    # dbg.py flips this for intermediate dumps


def build_nc(T=2048, V=DIM, F=512, mmdt=BF16):
    H = F // D                   # heads per core (8)
    KC = V // 128                # contraction chunks for projections (8)
    NT = T // 128                # 128-row tiles of T (16)
    TQ = 512                     # q-block width
    NJ = T // TQ                 # q-blocks (4)
    NS = T // 128                # key tiles (16)
    KF = F // 128                # ctx feature chunks (4)

    nc = bacc.Bacc(trn_type="TRN2")
    xT = nc.dram_tensor("xT", [V, T], mmdt, kind="ExternalInput")
    wqT = nc.dram_tensor("wqT", [V, F], mmdt, kind="ExternalInput")
    wkT = nc.dram_tensor("wkT", [V, F], mmdt, kind="ExternalInput")
    wvT = nc.dram_tensor("wvT", [V, F], mmdt, kind="ExternalInput")
    woT = nc.dram_tensor("woT", [F, V], mmdt, kind="ExternalInput")
    out = nc.dram_tensor("out", [T, V], F32, kind="ExternalOutput")
    if DEBUG:
        q_dbg = nc.dram_tensor("q_dbg", [128, T], mmdt, kind="ExternalOutput")
        k_dbg = nc.dram_tensor("k_dbg", [128, T], mmdt, kind="ExternalOutput")
        va_dbg = nc.dram_tensor("va_dbg", [128, 2048], mmdt, kind="ExternalOutput")
        ctx_dbg = nc.dram_tensor("ctx_dbg", [128, T], mmdt, kind="ExternalOutput")

    with tile.TileContext(nc) as tc:
        with (
            tc.tile_pool(name="pers", bufs=1) as pp,
            tc.tile_pool(name="exp", bufs=1) as pexp,
            tc.tile_pool(name="rz", bufs=1) as prz,
            tc.tile_pool(name="ot", bufs=1) as pot,
            tc.tile_pool(name="ps", bufs=1, space="PSUM") as ps,
        ):
            # ---------------- persistent SBUF ----------------
            xTs = [pp.tile([128, T], mmdt, tag=f"xT{k}", name=f"xTs{k}") for k in range(KC)]
            for k in range(KC):
                nc.sync.dma_start(xTs[k][:], xT[128 * k : 128 * (k + 1), :])

            def load_w(dram, nm, cols):
                ws = []
                for k in range(dram.shape[0] // 128):
                    w = pp.tile([128, cols], mmdt, tag=f"{nm}{k}", name=f"{nm}{k}")
                    nc.sync.dma_start(w[:], dram[128 * k : 128 * (k + 1), :])
                    ws.append(w)
                return ws

            wvs = load_w(wvT, "wv", F)
            wqs = load_w(wqT, "wq", F)
            wks = load_w(wkT, "wk", F)
            wos = load_w(woT, "wo", V)

            qT = [pp.tile([128, T], mmdt, tag=f"qT{m}", name=f"qT{m}") for m in range(KF)]
            kT = [pp.tile([128, T], mmdt, tag=f"kT{m}", name=f"kT{m}") for m in range(KF)]
            ctxT = [pp.tile([128, T], mmdt, tag=f"cT{m}", name=f"cT{m}") for m in range(KF)]
            # vaug: per key-tile t, per head h: [v(64) | 1 | 0*63] at cols 1024*t+128*h
            vaug = pp.tile([128, NT * 128 * H], mmdt, tag="vaug", name="vaug")
            nc.vector.memset(vaug[:], 0.0)
            nc.vector.memset(vaug[:, 64::128], 1.0)

            # ---------------- emission helpers ----------------
            def fill_tile():
                return ps.tile([128, 512], F32, tag="fill", bufs=1, name="fl")

            def sc_tile():
                return ps.tile([128, 1024], F32, tag="sc", bufs=2, name="sc")

            def v_tile(t):
                f = fill_tile()
                for k in range(KC):
                    nc.tensor.matmul(
                        f[:],
                        xTs[k][:, 128 * t : 128 * (t + 1)],
                        wvs[k][:],
                        start=(k == 0),
                        stop=(k == KC - 1),
                    )
                for h in range(H):
                    nc.vector.tensor_copy(
                        vaug[:, 1024 * t + 128 * h : 1024 * t + 128 * h + 64],
                        f[:, 64 * h : 64 * (h + 1)],
                    )

            def proj_blk(ws, dst, m, blk):
                # dst[m][:, 512*blk : 512*(blk+1)] = W_m chunk @ x^T T-block
                f = fill_tile()
                for k in range(KC):
                    nc.tensor.matmul(
                        f[:],
                        ws[k][:, 128 * m : 128 * (m + 1)],
                        xTs[k][:, 512 * blk : 512 * (blk + 1)],
                        start=(k == 0),
                        stop=(k == KC - 1),
                    )
                nc.vector.tensor_copy(dst[m][:, 512 * blk : 512 * (blk + 1)], f[:])

            def out_half(t, n):
                # out rows 128t..128t+128, cols 512n..512n+512
                f = fill_tile()
                for m in range(KF):
                    nc.tensor.matmul(
                        f[:],
                        ctxT[m][:, 128 * t : 128 * (t + 1)],
                        wos[m][:, 512 * n : 512 * (n + 1)],
                        start=(m == 0),
                        stop=(m == KF - 1),
                    )
                o = pot.tile([128, 512], F32, tag="ot", bufs=2, name="ot")
                nc.vector.tensor_copy(o[:], f[:])
                nc.sync.dma_start(out[128 * t : 128 * (t + 1), 512 * n : 512 * (n + 1)], o[:])

            def attn_group(j, p, fillers):
                """Attention for head pair p (heads 2p, 2p+1), q-block j.
                fillers: callables interleaved evenly into the s-loop."""
                pcx = [
                    ps.tile([128, 512], F32, tag="cx", bufs=3, name="pcx")
                    for _ in range(2)
                ]
                nf = len(fillers)
                emitted = 0
                for s in range(NS):
                    sc = sc_tile()
                    for half in range(2):
                        lo = 64 * half
                        nc.tensor.matmul(
                            sc[:, 512 * half : 512 * (half + 1)],
                            kT[p][lo : lo + 64, 128 * s : 128 * (s + 1)],
                            qT[p][lo : lo + 64, TQ * j : TQ * (j + 1)],
                            tile_position=(lo, 0),
                        )
                    e = pexp.tile([128, 1024], mmdt, tag="e", bufs=3, name="e")
                    if s in DVE_S:
                        nc.vector.tensor_scalar(
                            e[:].bitcast(I16), sc[:], EXP_A, EXP_B, ALU.mult, ALU.add
                        )
                    else:
                        nc.scalar.activation(e[:], sc[:], AF.Exp, scale=0.125)
                    for half in range(2):
                        h = 2 * p + half
                        nc.tensor.matmul(
                            pcx[half][:],
                            vaug[:, 1024 * s + 128 * h : 1024 * s + 128 * (h + 1)],
                            e[:, 512 * half : 512 * (half + 1)],
                            start=(s == 0),
                            stop=(s == NS - 1),
                        )
                    want = (s + 1) * nf // NS
                    while emitted < want:
                        fillers[emitted]()
                        emitted += 1
                for half in range(2):
                    lo = 64 * half
                    zs = prz.tile([1, 512], F32, tag="zs", bufs=2, name="zs")
                    nc.scalar.copy(zs[:], pcx[half][64:65, :])
                    rcp = prz.tile([1, 512], F32, tag="rcp", bufs=2, name="rcp")
                    nc.vector.reciprocal_approx_fast(rcp[:], zs[:])
                    bcs = prz.tile([64, 512], F32, tag="bcs", bufs=2, name="bcs")
                    nc.gpsimd.partition_broadcast(bcs[:], rcp[:])
                    nc.vector.tensor_mul(
                        ctxT[p][lo : lo + 64, TQ * j : TQ * (j + 1)],
                        pcx[half][0:64, :],
                        bcs[:],
                    )

            # ---------------- emission schedule ----------------
            # upfront: v tiles 0,1, all of kT[0], qT[0] block 0
            for t in range(2):
                v_tile(t)
            for blk in range(NJ):
                proj_blk(wks, kT, 0, blk)
            proj_blk(wqs, qT, 0, 0)

            # filler queues per (p, j) group:
            #   (0,0): v tiles 2..15 (tile s+2 lands at iter s) + qT[0] blks 1,2
            #   (0,1): qT[0] blk 3, then chunk-1 projections
            #   rest of p=0, p=1, p=2: chunk p+1 projections (ready before p+1)
            fq = {(p, j): [] for p in range(3) for j in range(NJ)}
            fq[(0, 0)] = [(lambda t=t: v_tile(t)) for t in range(2, NT)] + [
                lambda: proj_blk(wqs, qT, 0, 1),
                lambda: proj_blk(wqs, qT, 0, 2),
            ]
            fq[(0, 1)].append(lambda: proj_blk(wqs, qT, 0, 3))
            for p in range(3):
                m = p + 1
                chunk = [(lambda b=b, m=m: proj_blk(wks, kT, m, b)) for b in range(NJ)]
                chunk += [(lambda b=b, m=m: proj_blk(wqs, qT, m, b)) for b in range(NJ)]
                slots = [(0, 1), (0, 2), (0, 3)] if p == 0 else [(p, j) for j in range(NJ)]
                for i, f in enumerate(chunk):
                    fq[slots[i % len(slots)]].append(f)

            for p in range(3):
                for j in range(NJ):
                    attn_group(j, p, fq[(p, j)])
            # p=3: out-proj row tiles of block j-1 fill during block j
            for j in range(NJ):
                of = []
                if j >= 1:
                    jj = j - 1
                    of = [
                        (lambda t=t, n=n: out_half(t, n))
                        for t in range(4 * jj, 4 * jj + 4)
                        for n in range(2)
                    ]
                attn_group(j, 3, of)
            for t in range(12, NT):
                for n in range(2):
                    out_half(t, n)

            if DEBUG:
                nc.sync.dma_start(q_dbg[:, :], qT[0][:])
                nc.sync.dma_start(k_dbg[:, :], kT[0][:])
                nc.sync.dma_start(va_dbg[:, :], vaug[:, 0:2048])
                nc.sync.dma_start(ctx_dbg[:, :], ctxT[0][:])

    nc.compile()
    return nc


_NC_CACHE = {}


def _get_nc(T=2048, V=DIM, F=512):
    key = (T, V, F)
    if key not in _NC_CACHE:
        _NC_CACHE[key] = build_nc(T, V, F)
    return _NC_CACHE[key]


def make_in_maps(x, Wq, Wk, Wv, Wo, np_mmdt):
    B = x.shape[0]
    F = Wq.shape[0] // 2
    in_maps = []
    for c in range(N_CORES):
        b, g = divmod(c, 2)
        rows = slice(g * F, (g + 1) * F)
        in_maps.append(
            {
                "xT": np.ascontiguousarray(x[b].T).astype(np_mmdt),
                "wqT": np.ascontiguousarray(Wq[rows].T).astype(np_mmdt),
                "wkT": np.ascontiguousarray(Wk[rows].T).astype(np_mmdt),
                "wvT": np.ascontiguousarray(Wv[rows].T).astype(np_mmdt),
                "woT": np.ascontiguousarray(Wo[:, rows].T).astype(np_mmdt),
            }
        )
    return in_maps


def kernel(x, Wq, Wk, Wv, Wo, bo, trace=False):
    x = np.asarray(x, np.float32)
    B, T, V = x.shape
    nc = _get_nc(T=T, V=V, F=V // 2)
    np_mmdt = mybir.dt.np(BF16)
    in_maps = make_in_maps(
        x,
        np.asarray(Wq, np.float32),
        np.asarray(Wk, np.float32),
        np.asarray(Wv, np.float32),
        np.asarray(Wo, np.float32),
        np_mmdt,
    )
    res = run_bass_kernel_spmd(nc, in_maps, core_ids=list(range(N_CORES)), trace=trace)
    outs = [r["out"] for r in res.results]
    full = np.empty((B, T, V), np.float32)
    for b in range(B):
        full[b] = outs[2 * b] + outs[2 * b + 1] + np.asarray(bo, np.float32)
    if trace:
        kernel.last_exec_time_ns = res.exec_time_ns
        kernel.last_results = res
    return full


# revision 15
# speedup vs baseline: 1.1526x; 1.1526x over previous
"""Trainium2 Bass kernel for MiniSelfAttention (v2).

Shapes (full problem): x (4, 2048, 1024), Wq/Wk/Wv/Wo (1024, 1024), bo (1024,).
H=16 heads, D=64. out = softmax(q k^T / 8) v  projected by Wo.

Sharding across 8 cores: core c -> batch b = c//2, head-group g = c%2
(8 heads = 512 features per group). Each core computes a partial output
projection (its 512 ctx features x Wo slice); host sums the two partials
per batch and adds the bias.

v2 structure (vs v1 baseline):
  - Single PSUM region, 8 banks: 'sc' [128,1024]f32 x2 (scores, double
    buffered, 4 banks), 'cx' [128,512]f32 x3 (AV accumulators), 'fill'
    [128,512]f32 x1 (projection / v / out-proj tiles).
  - Loop order: head-pair OUTER, q-block (j) INNER. Projections for
    chunk m+1, v tiles, and out-proj row tiles are interleaved into the
    attention s-loops as PE "filler" so the PE stays dense (HAM warm)
    and the serial phase-1 head is minimized.
  - exp split between ScalarE (true exp) and VectorE (Schraudolph
    bit-trick: bf16_bits = round_i16(A*score + B), ~2-3% sawtooth error
    that cancels in softmax) to relieve the ScalarE bottleneck.
  - softmax: Z row (ones column in vaug) -> ScalarE copy to SBUF ->
    DVE reciprocal_approx_fast -> gpsimd partition_broadcast -> one
    fused tensor_mul into ctxT. (reciprocal_approx_fast must NOT read
    PSUM directly - it misreads partition-offset rows of bank-offset
    PSUM tiles.)
"""

import sys

sys.path.insert(0, "/opt/trn_rl_repo")

import numpy as np

import concourse.bacc as bacc
import concourse.mybir as mybir
from concourse import tile
from concourse.bass_utils import run_bass_kernel_spmd

F32 = mybir.dt.float32
BF16 = mybir.dt.bfloat16
I16 = mybir.dt.int16
AF = mybir.ActivationFunctionType
ALU = mybir.AluOpType

DIM = 1024
HEADS = 16
D = 64
N_CORES = 8

LOG2E = 1.4426950408889634
EXP_A = 128.0 * LOG2E / 8.0      # folds the 1/sqrt(D) score scale
EXP_B = 16248.6                  # 127*128 tuned: centers the sawtooth error
DVE_S = (1, 5, 9, 13)            # s-tiles whose exp runs on VectorE

DEBUG = False                    # dbg.py flips this for intermediate dumps


def build_nc(T=2048, V=DIM, F=512, mmdt=BF16):
    H = F // D                   # heads per core (8)
    KC = V // 128                # contraction chunks for projections (8)
    NT = T // 128                # 128-row tiles of T (16)
    TQ = 512                     # q-block width
    NJ = T // TQ                 # q-blocks (4)
    NS = T // 128                # key tiles (16)
    KF = F // 128                # ctx feature chunks (4)

    nc = bacc.Bacc(trn_type="TRN2")
    xT = nc.dram_tensor("xT", [V, T], mmdt, kind="ExternalInput")
    wqT = nc.dram_tensor("wqT", [V, F], mmdt, kind="ExternalInput")
    wkT = nc.dram_tensor("wkT", [V, F], mmdt, kind="ExternalInput")
    wvT = nc.dram_tensor("wvT", [V, F], mmdt, kind="ExternalInput")
    woT = nc.dram_tensor("woT", [F, V], mmdt, kind="ExternalInput")
    out = nc.dram_tensor("out", [T, V], F32, kind="ExternalOutput")
    if DEBUG:
        q_dbg = nc.dram_tensor("q_dbg", [128, T], mmdt, kind="ExternalOutput")
        k_dbg = nc.dram_tensor("k_dbg", [128, T], mmdt, kind="ExternalOutput")
        va_dbg = nc.dram_tensor("va_dbg", [128, 2048], mmdt, kind="ExternalOutput")
        ctx_dbg = nc.dram_tensor("ctx_dbg", [128, T], mmdt, kind="ExternalOutput")

    with tile.TileContext(nc) as tc:
        with (
            tc.tile_pool(name="pers", bufs=1) as pp,
            tc.tile_pool(name="exp", bufs=1) as pexp,
            tc.tile_pool(name="rz", bufs=1) as prz,
            tc.tile_pool(name="ot", bufs=1) as pot,
            tc.tile_pool(name="ps", bufs=1, space="PSUM") as ps,
        ):
            # ---------------- persistent SBUF ----------------
            xTs = [pp.tile([128, T], mmdt, tag=f"xT{k}", name=f"xTs{k}") for k in range(KC)]
            for k in range(KC):
                nc.sync.dma_start(xTs[k][:], xT[128 * k : 128 * (k + 1), :])

            def load_w(dram, nm, cols):
                ws = []
                for k in range(dram.shape[0] // 128):
                    w = pp.tile([128, cols], mmdt, tag=f"{nm}{k}", name=f"{nm}{k}")
                    nc.sync.dma_start(w[:], dram[128 * k : 128 * (k + 1), :])
                    ws.append(w)
                return ws

            wvs = load_w(wvT, "wv", F)
            wqs = load_w(wqT, "wq", F)
            wks = load_w(wkT, "wk", F)
            wos = load_w(woT, "wo", V)

            qT = [pp.tile([128, T], mmdt, tag=f"qT{m}", name=f"qT{m}") for m in range(KF)]
            kT = [pp.tile([128, T], mmdt, tag=f"kT{m}", name=f"kT{m}") for m in range(KF)]
            ctxT = [pp.tile([128, T], mmdt, tag=f"cT{m}", name=f"cT{m}") for m in range(KF)]
            # vaug: per key-tile t, per head h: [v(64) | 1 | 0*63] at cols 1024*t+128*h
            vaug = pp.tile([128, NT * 128 * H], mmdt, tag="vaug", name="vaug")
            nc.vector.memset(vaug[:], 0.0)
            nc.vector.memset(vaug[:, 64::128], 1.0)

            # ---------------- emission helpers ----------------
            def fill_tile():
                return ps.tile([128, 512], F32, tag="fill", bufs=1, name="fl")

            def sc_tile():
                return ps.tile([128, 1024], F32, tag="sc", bufs=2, name="sc")

            def v_tile(t, ce="vector"):
                f = fill_tile()
                for k in range(KC):
                    nc.tensor.matmul(
                        f[:],
                        xTs[k][:, 128 * t : 128 * (t + 1)],
                        wvs[k][:],
                        start=(k == 0),
                        stop=(k == KC - 1),
                    )
                eng = nc.vector if ce == "vector" else nc.scalar
                for h in range(H):
                    cp = (nc.vector.tensor_copy if ce == "vector" else nc.scalar.copy)
                    cp(
                        vaug[:, 1024 * t + 128 * h : 1024 * t + 128 * h + 64],
                        f[:, 64 * h : 64 * (h + 1)],
                    )

            def proj_blk(ws, dst, m, blk, ce="vector"):
                # dst[m][:, 512*blk : 512*(blk+1)] = W_m chunk @ x^T T-block
                f = fill_tile()
                for k in range(KC):
                    nc.tensor.matmul(
                        f[:],
                        ws[k][:, 128 * m : 128 * (m + 1)],
                        xTs[k][:, 512 * blk : 512 * (blk + 1)],
                        start=(k == 0),
                        stop=(k == KC - 1),
                    )
                cp = nc.vector.tensor_copy if ce == "vector" else nc.scalar.copy
                cp(dst[m][:, 512 * blk : 512 * (blk + 1)], f[:])

            def out_half(t, n, ce="vector"):
                # out rows 128t..128t+128, cols 512n..512n+512
                f = fill_tile()
                for m in range(KF):
                    nc.tensor.matmul(
                        f[:],
                        ctxT[m][:, 128 * t : 128 * (t + 1)],
                        wos[m][:, 512 * n : 512 * (n + 1)],
                        start=(m == 0),
                        stop=(m == KF - 1),
                    )
                o = pot.tile([128, 512], F32, tag="ot", bufs=2, name="ot")
                cp = nc.vector.tensor_copy if ce == "vector" else nc.scalar.copy
                cp(o[:], f[:])
                nc.sync.dma_start(out[128 * t : 128 * (t + 1), 512 * n : 512 * (n + 1)], o[:])

            def attn_group(j, p, fillers):
                """Attention for head pair p (heads 2p, 2p+1), q-block j.
                fillers: callables interleaved evenly into the s-loop."""
                pcx = [
                    ps.tile([128, 512], F32, tag="cx", bufs=3, name="pcx")
                    for _ in range(2)
                ]
                nf = len(fillers)
                emitted = 0
                for s in range(NS):
                    sc = sc_tile()
                    for half in range(2):
                        lo = 64 * half
                        nc.tensor.matmul(
                            sc[:, 512 * half : 512 * (half + 1)],
                            kT[p][lo : lo + 64, 128 * s : 128 * (s + 1)],
                            qT[p][lo : lo + 64, TQ * j : TQ * (j + 1)],
                            tile_position=(lo, 0),
                        )
                    e = pexp.tile([128, 1024], mmdt, tag="e", bufs=3, name="e")
                    if s in DVE_S:
                        nc.vector.tensor_scalar(
                            e[:].bitcast(I16), sc[:], EXP_A, EXP_B, ALU.mult, ALU.add
                        )
                    else:
                        nc.scalar.activation(e[:], sc[:], AF.Exp, scale=0.125)
                    for half in range(2):
                        h = 2 * p + half
                        nc.tensor.matmul(
                            pcx[half][:],
                            vaug[:, 1024 * s + 128 * h : 1024 * s + 128 * (h + 1)],
                            e[:, 512 * half : 512 * (half + 1)],
                            start=(s == 0),
                            stop=(s == NS - 1),
                        )
                    want = (s + 1) * nf // NS
                    ce = "scalar" if s in DVE_S else "vector"
                    while emitted < want:
                        fillers[emitted](ce)
                        emitted += 1
                for half in range(2):
                    lo = 64 * half
                    zs = prz.tile([1, 512], F32, tag="zs", bufs=2, name="zs")
                    nc.scalar.copy(zs[:], pcx[half][64:65, :])
                    rcp = prz.tile([1, 512], F32, tag="rcp", bufs=2, name="rcp")
                    nc.vector.reciprocal_approx_fast(rcp[:], zs[:])
                    bcs = prz.tile([64, 512], F32, tag="bcs", bufs=2, name="bcs")
                    nc.gpsimd.partition_broadcast(bcs[:], rcp[:])
                    nc.vector.tensor_mul(
                        ctxT[p][lo : lo + 64, TQ * j : TQ * (j + 1)],
                        pcx[half][0:64, :],
                        bcs[:],
                    )

            # ---------------- emission schedule ----------------
            # upfront: v tiles 0,1, all of kT[0], qT[0] block 0
            for t in range(2):
                v_tile(t)
            for blk in range(NJ):
                proj_blk(wks, kT, 0, blk)
            proj_blk(wqs, qT, 0, 0)

            # filler queues per (p, j) group:
            #   (0,0): v tiles 2..15 (tile s+2 lands at iter s) + qT[0] blks 1,2
            #   (0,1): qT[0] blk 3, then chunk-1 projections
            #   rest of p=0, p=1, p=2: chunk p+1 projections (ready before p+1)
            fq = {(p, j): [] for p in range(3) for j in range(NJ)}
            fq[(0, 0)] = [(lambda ce, t=t: v_tile(t, ce)) for t in range(2, NT)] + [
                lambda ce: proj_blk(wqs, qT, 0, 1, ce),
                lambda ce: proj_blk(wqs, qT, 0, 2, ce),
            ]
            fq[(0, 1)].append(lambda ce: proj_blk(wqs, qT, 0, 3, ce))
            for p in range(3):
                m = p + 1
                chunk = [(lambda ce, b=b, m=m: proj_blk(wks, kT, m, b, ce)) for b in range(NJ)]
                chunk += [(lambda ce, b=b, m=m: proj_blk(wqs, qT, m, b, ce)) for b in range(NJ)]
                slots = [(0, 1), (0, 2), (0, 3)] if p == 0 else [(p, j) for j in range(NJ)]
                for i, f in enumerate(chunk):
                    fq[slots[i % len(slots)]].append(f)

            for p in range(3):
                for j in range(NJ):
                    attn_group(j, p, fq[(p, j)])
            # p=3: out-proj row tiles of block j-1 fill during block j
            for j in range(NJ):
                of = []
                if j >= 1:
                    jj = j - 1
                    of = [
                        (lambda ce, t=t, n=n: out_half(t, n, ce))
                        for t in range(4 * jj, 4 * jj + 4)
                        for n in range(2)
                    ]
                attn_group(j, 3, of)
            for t in range(12, NT):
                for n in range(2):
                    out_half(t, n)

            if DEBUG:
                nc.sync.dma_start(q_dbg[:, :], qT[0][:])
                nc.sync.dma_start(k_dbg[:, :], kT[0][:])
                nc.sync.dma_start(va_dbg[:, :], vaug[:, 0:2048])
                nc.sync.dma_start(ctx_dbg[:, :], ctxT[0][:])

    nc.compile()
    return nc


_NC_CACHE = {}


def _get_nc(T=2048, V=DIM, F=512):
    key = (T, V, F)
    if key not in _NC_CACHE:
        _NC_CACHE[key] = build_nc(T, V, F)
    return _NC_CACHE[key]


def make_in_maps(x, Wq, Wk, Wv, Wo, np_mmdt):
    B = x.shape[0]
    F = Wq.shape[0] // 2
    in_maps = []
    for c in range(N_CORES):
        b, g = divmod(c, 2)
        rows = slice(g * F, (g + 1) * F)
        in_maps.append(
            {
                "xT": np.ascontiguousarray(x[b].T).astype(np_mmdt),
                "wqT": np.ascontiguousarray(Wq[rows].T).astype(np_mmdt),
                "wkT": np.ascontiguousarray(Wk[rows].T).astype(np_mmdt),
                "wvT": np.ascontiguousarray(Wv[rows].T).astype(np_mmdt),
                "woT": np.ascontiguousarray(Wo[:, rows].T).astype(np_mmdt),
            }
        )
    return in_maps


def kernel(x, Wq, Wk, Wv, Wo, bo, trace=False):
    x = np.asarray(x, np.float32)
    B, T, V = x.shape
    nc = _get_nc(T=T, V=V, F=V // 2)
    np_mmdt = mybir.dt.np(BF16)
    in_maps = make_in_maps(
        x,
        np.asarray(Wq, np.float32),
        np.asarray(Wk, np.float32),
        np.asarray(Wv, np.float32),
        np.asarray(Wo, np.float32),
        np_mmdt,
    )
    res = run_bass_kernel_spmd(nc, in_maps, core_ids=list(range(N_CORES)), trace=trace)
    outs = [r["out"] for r in res.results]
    full = np.empty((B, T, V), np.float32)
    for b in range(B):
        full[b] = outs[2 * b] + outs[2 * b + 1] + np.asarray(bo, np.float32)
    if trace:
        kernel.last_exec_time_ns = res.exec_time_ns
        kernel.last_results = res
    return full


# revision 17
# speedup vs baseline: 1.1658x; 1.0115x over previous
"""Trainium2 Bass kernel for MiniSelfAttention (v2).

Shapes (full problem): x (4, 2048, 1024), Wq/Wk/Wv/Wo (1024, 1024), bo (1024,).
H=16 heads, D=64. out = softmax(q k^T / 8) v  projected by Wo.

Sharding across 8 cores: core c -> batch b = c//2, head-group g = c%2
(8 heads = 512 features per group). Each core computes a partial output
projection (its 512 ctx features x Wo slice); host sums the two partials
per batch and adds the bias.

v2 structure (vs v1 baseline):
  - Single PSUM region, 8 banks: 'sc' [128,1024]f32 x2 (scores, double
    buffered, 4 banks), 'cx' [128,512]f32 x3 (AV accumulators), 'fill'
    [128,512]f32 x1 (projection / v / out-proj tiles).
  - Loop order: head-pair OUTER, q-block (j) INNER. Projections for
    chunk m+1, v tiles, and out-proj row tiles are interleaved into the
    attention s-loops as PE "filler" so the PE stays dense (HAM warm)
    and the serial phase-1 head is minimized.
  - exp split between ScalarE (true exp) and VectorE (Schraudolph
    bit-trick: bf16_bits = round_i16(A*score + B), ~2-3% sawtooth error
    that cancels in softmax) to relieve the ScalarE bottleneck.
  - softmax: Z row (ones column in vaug) -> ScalarE copy to SBUF ->
    DVE reciprocal_approx_fast -> gpsimd partition_broadcast -> one
    fused tensor_mul into ctxT. (reciprocal_approx_fast must NOT read
    PSUM directly - it misreads partition-offset rows of bank-offset
    PSUM tiles.)
"""

import sys

sys.path.insert(0, "/opt/trn_rl_repo")

import numpy as np

import concourse.bacc as bacc
import concourse.mybir as mybir
from concourse import tile
from concourse.bass_utils import run_bass_kernel_spmd

F32 = mybir.dt.float32
BF16 = mybir.dt.bfloat16
I16 = mybir.dt.int16
AF = mybir.ActivationFunctionType
ALU = mybir.AluOpType

DIM = 1024
HEADS = 16
D = 64
N_CORES = 8

LOG2E = 1.4426950408889634
EXP_A = 128.0 * LOG2E / 8.0      # folds the 1/sqrt(D) score scale
EXP_B = 16248.6                  # 127*128 tuned: centers the sawtooth error
DVE_S = (1, 4, 7, 10, 13)        # s-tiles whose exp runs on VectorE

DEBUG = False                    # dbg.py flips this for intermediate dumps


def build_nc(T=2048, V=DIM, F=512, mmdt=BF16):
    H = F // D                   # heads per core (8)
    KC = V // 128                # contraction chunks for projections (8)
    NT = T // 128                # 128-row tiles of T (16)
    TQ = 512                     # q-block width
    NJ = T // TQ                 # q-blocks (4)
    NS = T // 128                # key tiles (16)
    KF = F // 128                # ctx feature chunks (4)

    nc = bacc.Bacc(trn_type="TRN2")
    xT = nc.dram_tensor("xT", [V, T], mmdt, kind="ExternalInput")
    wqT = nc.dram_tensor("wqT", [V, F], mmdt, kind="ExternalInput")
    wkT = nc.dram_tensor("wkT", [V, F], mmdt, kind="ExternalInput")
    wvT = nc.dram_tensor("wvT", [V, F], mmdt, kind="ExternalInput")
    woT = nc.dram_tensor("woT", [F, V], mmdt, kind="ExternalInput")
    out = nc.dram_tensor("out", [T, V], F32, kind="ExternalOutput")
    if DEBUG:
        q_dbg = nc.dram_tensor("q_dbg", [128, T], mmdt, kind="ExternalOutput")
        k_dbg = nc.dram_tensor("k_dbg", [128, T], mmdt, kind="ExternalOutput")
        va_dbg = nc.dram_tensor("va_dbg", [128, 2048], mmdt, kind="ExternalOutput")
        ctx_dbg = nc.dram_tensor("ctx_dbg", [128, T], mmdt, kind="ExternalOutput")

    with tile.TileContext(nc) as tc:
        with (
            tc.tile_pool(name="pers", bufs=1) as pp,
            tc.tile_pool(name="exp", bufs=1) as pexp,
            tc.tile_pool(name="rz", bufs=1) as prz,
            tc.tile_pool(name="ot", bufs=1) as pot,
            tc.tile_pool(name="ps", bufs=1, space="PSUM") as ps,
        ):
            # ---------------- persistent SBUF ----------------
            xTs = [pp.tile([128, T], mmdt, tag=f"xT{k}", name=f"xTs{k}") for k in range(KC)]
            for k in range(KC):
                nc.sync.dma_start(xTs[k][:], xT[128 * k : 128 * (k + 1), :])

            def load_w(dram, nm, cols):
                ws = []
                for k in range(dram.shape[0] // 128):
                    w = pp.tile([128, cols], mmdt, tag=f"{nm}{k}", name=f"{nm}{k}")
                    nc.sync.dma_start(w[:], dram[128 * k : 128 * (k + 1), :])
                    ws.append(w)
                return ws

            wvs = load_w(wvT, "wv", F)
            wqs = load_w(wqT, "wq", F)
            wks = load_w(wkT, "wk", F)
            wos = load_w(woT, "wo", V)

            qT = [pp.tile([128, T], mmdt, tag=f"qT{m}", name=f"qT{m}") for m in range(KF)]
            kT = [pp.tile([128, T], mmdt, tag=f"kT{m}", name=f"kT{m}") for m in range(KF)]
            ctxT = [pp.tile([128, T], mmdt, tag=f"cT{m}", name=f"cT{m}") for m in range(KF)]
            # vaug: per key-tile t, per head h: [v(64) | 1 | 0*63] at cols 1024*t+128*h
            vaug = pp.tile([128, NT * 128 * H], mmdt, tag="vaug", name="vaug")
            nc.vector.memset(vaug[:], 0.0)
            nc.vector.memset(vaug[:, 64::128], 1.0)

            # ---------------- emission helpers ----------------
            def fill_tile():
                return ps.tile([128, 512], F32, tag="fill", bufs=1, name="fl")

            def sc_tile():
                return ps.tile([128, 1024], F32, tag="sc", bufs=2, name="sc")

            def v_tile(t, ce="vector"):
                f = fill_tile()
                for k in range(KC):
                    nc.tensor.matmul(
                        f[:],
                        xTs[k][:, 128 * t : 128 * (t + 1)],
                        wvs[k][:],
                        start=(k == 0),
                        stop=(k == KC - 1),
                    )
                eng = nc.vector if ce == "vector" else nc.scalar
                for h in range(H):
                    cp = (nc.vector.tensor_copy if ce == "vector" else nc.scalar.copy)
                    cp(
                        vaug[:, 1024 * t + 128 * h : 1024 * t + 128 * h + 64],
                        f[:, 64 * h : 64 * (h + 1)],
                    )

            def proj_blk(ws, dst, m, blk, ce="vector"):
                # dst[m][:, 512*blk : 512*(blk+1)] = W_m chunk @ x^T T-block
                f = fill_tile()
                for k in range(KC):
                    nc.tensor.matmul(
                        f[:],
                        ws[k][:, 128 * m : 128 * (m + 1)],
                        xTs[k][:, 512 * blk : 512 * (blk + 1)],
                        start=(k == 0),
                        stop=(k == KC - 1),
                    )
                cp = nc.vector.tensor_copy if ce == "vector" else nc.scalar.copy
                cp(dst[m][:, 512 * blk : 512 * (blk + 1)], f[:])

            def out_half(t, n, ce="vector", pool="fill"):
                # out rows 128t..128t+128, cols 512n..512n+512
                ft = fill_tile() if pool == "fill" else sc_tile()
                f = ft if pool == "fill" else ft[:, 0:512]
                for m in range(KF):
                    nc.tensor.matmul(
                        f[:],
                        ctxT[m][:, 128 * t : 128 * (t + 1)],
                        wos[m][:, 512 * n : 512 * (n + 1)],
                        start=(m == 0),
                        stop=(m == KF - 1),
                    )
                o = pot.tile([128, 512], F32, tag="ot", bufs=2, name="ot")
                cp = nc.vector.tensor_copy if ce == "vector" else nc.scalar.copy
                cp(o[:], f[:])
                nc.sync.dma_start(out[128 * t : 128 * (t + 1), 512 * n : 512 * (n + 1)], o[:])

            def attn_group(j, p, fillers):
                """Attention for head pair p (heads 2p, 2p+1), q-block j.
                fillers: callables interleaved evenly into the s-loop."""
                pcx = [
                    ps.tile([128, 512], F32, tag="cx", bufs=3, name="pcx")
                    for _ in range(2)
                ]
                nf = len(fillers)
                emitted = 0
                es = [None] * NS

                def av(s):
                    for half in range(2):
                        h = 2 * p + half
                        nc.tensor.matmul(
                            pcx[half][:],
                            vaug[:, 1024 * s + 128 * h : 1024 * s + 128 * (h + 1)],
                            es[s][:, 512 * half : 512 * (half + 1)],
                            start=(s == 0),
                            stop=(s == NS - 1),
                        )

                for s in range(NS):
                    sc = sc_tile()
                    for half in range(2):
                        lo = 64 * half
                        nc.tensor.matmul(
                            sc[:, 512 * half : 512 * (half + 1)],
                            kT[p][lo : lo + 64, 128 * s : 128 * (s + 1)],
                            qT[p][lo : lo + 64, TQ * j : TQ * (j + 1)],
                            tile_position=(lo, 0),
                        )
                    es[s] = pexp.tile([128, 1024], mmdt, tag="e", bufs=3, name="e")
                    if s in DVE_S:
                        nc.vector.tensor_scalar(
                            es[s][:].bitcast(I16), sc[:], EXP_A, EXP_B, ALU.mult, ALU.add
                        )
                    else:
                        nc.scalar.activation(es[s][:], sc[:], AF.Exp, scale=0.125)
                    if s >= 1:
                        av(s - 1)
                    want = (s + 1) * nf // NS
                    ce = "scalar" if s in DVE_S else "vector"
                    while emitted < want:
                        fillers[emitted](ce)
                        emitted += 1
                av(NS - 1)
                for half in range(2):
                    lo = 64 * half
                    zs = prz.tile([1, 512], F32, tag="zs", bufs=2, name="zs")
                    nc.scalar.copy(zs[:], pcx[half][64:65, :])
                    rcp = prz.tile([1, 512], F32, tag="rcp", bufs=2, name="rcp")
                    nc.vector.reciprocal_approx_fast(rcp[:], zs[:])
                    bcs = prz.tile([64, 512], F32, tag="bcs", bufs=2, name="bcs")
                    nc.gpsimd.partition_broadcast(bcs[:], rcp[:])
                    nc.vector.tensor_mul(
                        ctxT[p][lo : lo + 64, TQ * j : TQ * (j + 1)],
                        pcx[half][0:64, :],
                        bcs[:],
                    )

            # ---------------- emission schedule ----------------
            # upfront: minimal gate for attn(0,0): kT[0] blk0, qT[0] blk0,
            # v tiles 0..3 (v t lands JIT at iter t-2 for the rest)
            proj_blk(wks, kT, 0, 0)
            proj_blk(wqs, qT, 0, 0)
            for t in range(4):
                v_tile(t)

            # filler queues per (p, j) group; k0 blk b needed by iter 4b of
            # every p=0 group; v tile t needed at iter t of (0,0); q0 blk b
            # needed by group (0,b).
            fq = {(p, j): [] for p in range(3) for j in range(NJ)}
            f_v = lambda t: (lambda ce, t=t: v_tile(t, ce))
            f_p = lambda ws, dst, m, b: (lambda ce, m=m, b=b: proj_blk(ws, dst, m, b, ce))
            fq[(0, 0)] = [
                f_v(4), f_v(5), f_p(wks, kT, 0, 1), f_v(6), f_v(7),
                f_p(wks, kT, 0, 2), f_v(8), f_v(9), f_v(10),
                f_p(wks, kT, 0, 3), f_v(11), f_v(12), f_v(13), f_v(14),
                f_v(15), f_p(wqs, qT, 0, 1),
            ]
            fq[(0, 1)].append(f_p(wqs, qT, 0, 2))
            fq[(0, 2)].append(f_p(wqs, qT, 0, 3))
            for p in range(3):
                m = p + 1
                chunk = [f_p(wks, kT, m, b) for b in range(NJ)]
                chunk += [f_p(wqs, qT, m, b) for b in range(NJ)]
                slots = [(0, 1), (0, 2), (0, 3)] if p == 0 else [(p, j) for j in range(NJ)]
                for i, f in enumerate(chunk):
                    fq[slots[i % len(slots)]].append(f)

            for p in range(3):
                for j in range(NJ):
                    attn_group(j, p, fq[(p, j)])
            # p=3: out-proj row tiles of block j-1 fill during block j
            for j in range(NJ):
                of = []
                if j >= 1:
                    jj = j - 1
                    of = [
                        (lambda ce, t=t, n=n: out_half(t, n, ce))
                        for t in range(4 * jj, 4 * jj + 4)
                        for n in range(2)
                    ]
                attn_group(j, 3, of)
            for i, (t, n) in enumerate([(t, n) for t in range(12, NT) for n in range(2)]):
                out_half(t, n, pool=("fill" if i % 2 == 0 else "sc"))

            if DEBUG:
                nc.sync.dma_start(q_dbg[:, :], qT[0][:])
                nc.sync.dma_start(k_dbg[:, :], kT[0][:])
                nc.sync.dma_start(va_dbg[:, :], vaug[:, 0:2048])
                nc.sync.dma_start(ctx_dbg[:, :], ctxT[0][:])

    nc.compile()
    return nc


_NC_CACHE = {}


def _get_nc(T=2048, V=DIM, F=512):
    key = (T, V, F)
    if key not in _NC_CACHE:
        _NC_CACHE[key] = build_nc(T, V, F)
    return _NC_CACHE[key]


def make_in_maps(x, Wq, Wk, Wv, Wo, np_mmdt):
    B = x.shape[0]
    F = Wq.shape[0] // 2
    in_maps = []
    for c in range(N_CORES):
        b, g = divmod(c, 2)
        rows = slice(g * F, (g + 1) * F)
        in_maps.append(
            {
                "xT": np.ascontiguousarray(x[b].T).astype(np_mmdt),
                "wqT": np.ascontiguousarray(Wq[rows].T).astype(np_mmdt),
                "wkT": np.ascontiguousarray(Wk[rows].T).astype(np_mmdt),
                "wvT": np.ascontiguousarray(Wv[rows].T).astype(np_mmdt),
                "woT": np.ascontiguousarray(Wo[:, rows].T).astype(np_mmdt),
            }
        )
    return in_maps


def kernel(x, Wq, Wk, Wv, Wo, bo, trace=False):
    x = np.asarray(x, np.float32)
    B, T, V = x.shape
    nc = _get_nc(T=T, V=V, F=V // 2)
    np_mmdt = mybir.dt.np(BF16)
    in_maps = make_in_maps(
        x,
        np.asarray(Wq, np.float32),
        np.asarray(Wk, np.float32),
        np.asarray(Wv, np.float32),
        np.asarray(Wo, np.float32),
        np_mmdt,
    )
    res = run_bass_kernel_spmd(nc, in_maps, core_ids=list(range(N_CORES)), trace=trace)
    outs = [r["out"] for r in res.results]
    full = np.empty((B, T, V), np.float32)
    for b in range(B):
        full[b] = outs[2 * b] + outs[2 * b + 1] + np.asarray(bo, np.float32)
    if trace:
        kernel.last_exec_time_ns = res.exec_time_ns
        kernel.last_results = res
    return full


# revision 18
# speedup vs baseline: 1.1871x; 1.0182x over previous
"""Trainium2 Bass kernel for MiniSelfAttention (v2).

Shapes (full problem): x (4, 2048, 1024), Wq/Wk/Wv/Wo (1024, 1024), bo (1024,).
H=16 heads, D=64. out = softmax(q k^T / 8) v  projected by Wo.

Sharding across 8 cores: core c -> batch b = c//2, head-group g = c%2
(8 heads = 512 features per group). Each core computes a partial output
projection (its 512 ctx features x Wo slice); host sums the two partials
per batch and adds the bias.

v2 structure (vs v1 baseline):
  - Single PSUM region, 8 banks: 'sc' [128,1024]f32 x2 (scores, double
    buffered, 4 banks), 'cx' [128,512]f32 x3 (AV accumulators), 'fill'
    [128,512]f32 x1 (projection / v / out-proj tiles).
  - Loop order: head-pair OUTER, q-block (j) INNER. Projections for
    chunk m+1, v tiles, and out-proj row tiles are interleaved into the
    attention s-loops as PE "filler" so the PE stays dense (HAM warm)
    and the serial phase-1 head is minimized.
  - exp split between ScalarE (true exp) and VectorE (Schraudolph
    bit-trick: bf16_bits = round_i16(A*score + B), ~2-3% sawtooth error
    that cancels in softmax) to relieve the ScalarE bottleneck.
  - softmax: Z row (ones column in vaug) -> ScalarE copy to SBUF ->
    DVE reciprocal_approx_fast -> gpsimd partition_broadcast -> one
    fused tensor_mul into ctxT. (reciprocal_approx_fast must NOT read
    PSUM directly - it misreads partition-offset rows of bank-offset
    PSUM tiles.)
"""

import sys

sys.path.insert(0, "/opt/trn_rl_repo")

import numpy as np

import concourse.bacc as bacc
import concourse.mybir as mybir
from concourse import tile
from concourse.bass_utils import run_bass_kernel_spmd

F32 = mybir.dt.float32
BF16 = mybir.dt.bfloat16
I16 = mybir.dt.int16
AF = mybir.ActivationFunctionType
ALU = mybir.AluOpType

DIM = 1024
HEADS = 16
D = 64
N_CORES = 8

LOG2E = 1.4426950408889634
EXP_A = 128.0 * LOG2E / 8.0      # folds the 1/sqrt(D) score scale
EXP_B = 16248.6                  # 127*128 tuned: centers the sawtooth error
DVE_S = (1, 5, 9, 13)            # s-tiles whose exp runs on VectorE

DEBUG = False                    # dbg.py flips this for intermediate dumps


def build_nc(T=2048, V=DIM, F=512, mmdt=BF16):
    H = F // D                   # heads per core (8)
    KC = V // 128                # contraction chunks for projections (8)
    NT = T // 128                # 128-row tiles of T (16)
    TQ = 512                     # q-block width
    NJ = T // TQ                 # q-blocks (4)
    NS = T // 128                # key tiles (16)
    KF = F // 128                # ctx feature chunks (4)

    nc = bacc.Bacc(trn_type="TRN2")
    xT = nc.dram_tensor("xT", [V, T], mmdt, kind="ExternalInput")
    wqT = nc.dram_tensor("wqT", [V, F], mmdt, kind="ExternalInput")
    wkT = nc.dram_tensor("wkT", [V, F], mmdt, kind="ExternalInput")
    wvT = nc.dram_tensor("wvT", [V, F], mmdt, kind="ExternalInput")
    woT = nc.dram_tensor("woT", [F, V], mmdt, kind="ExternalInput")
    out = nc.dram_tensor("out", [T, V], F32, kind="ExternalOutput")
    if DEBUG:
        q_dbg = nc.dram_tensor("q_dbg", [128, T], mmdt, kind="ExternalOutput")
        k_dbg = nc.dram_tensor("k_dbg", [128, T], mmdt, kind="ExternalOutput")
        va_dbg = nc.dram_tensor("va_dbg", [128, 2048], mmdt, kind="ExternalOutput")
        ctx_dbg = nc.dram_tensor("ctx_dbg", [128, T], mmdt, kind="ExternalOutput")

    with tile.TileContext(nc) as tc:
        with (
            tc.tile_pool(name="pers", bufs=1) as pp,
            tc.tile_pool(name="exp", bufs=1) as pexp,
            tc.tile_pool(name="rz", bufs=1) as prz,
            tc.tile_pool(name="ot", bufs=1) as pot,
            tc.tile_pool(name="ps", bufs=1, space="PSUM") as ps,
        ):
            # ---------------- persistent SBUF ----------------
            xTs = [pp.tile([128, T], mmdt, tag=f"xT{k}", name=f"xTs{k}") for k in range(KC)]
            for k in range(KC):
                nc.sync.dma_start(xTs[k][:], xT[128 * k : 128 * (k + 1), :])

            def load_w(dram, nm, cols):
                ws = []
                for k in range(dram.shape[0] // 128):
                    w = pp.tile([128, cols], mmdt, tag=f"{nm}{k}", name=f"{nm}{k}")
                    nc.sync.dma_start(w[:], dram[128 * k : 128 * (k + 1), :])
                    ws.append(w)
                return ws

            wks = load_w(wkT, "wk", F)
            wqs = load_w(wqT, "wq", F)
            wvs = load_w(wvT, "wv", F)
            wos = load_w(woT, "wo", V)

            qT = [pp.tile([128, T], mmdt, tag=f"qT{m}", name=f"qT{m}") for m in range(KF)]
            kT = [pp.tile([128, T], mmdt, tag=f"kT{m}", name=f"kT{m}") for m in range(KF)]
            ctxT = [pp.tile([128, T], mmdt, tag=f"cT{m}", name=f"cT{m}") for m in range(KF)]
            # vaug: per key-tile t, per head h: [v(64) | 1 | 0*63] at cols 1024*t+128*h
            vaug = pp.tile([128, NT * 128 * H], mmdt, tag="vaug", name="vaug")
            nc.vector.memset(vaug[:], 0.0)
            nc.vector.memset(vaug[:, 64::128], 1.0)

            # ---------------- emission helpers ----------------
            def fill_tile():
                return ps.tile([128, 512], F32, tag="fill", bufs=1, name="fl")

            def sc_tile():
                return ps.tile([128, 1024], F32, tag="sc", bufs=2, name="sc")

            def v_tile(t, ce="vector"):
                f = fill_tile()
                for k in range(KC):
                    nc.tensor.matmul(
                        f[:],
                        xTs[k][:, 128 * t : 128 * (t + 1)],
                        wvs[k][:],
                        start=(k == 0),
                        stop=(k == KC - 1),
                    )
                eng = nc.vector if ce == "vector" else nc.scalar
                for h in range(H):
                    cp = (nc.vector.tensor_copy if ce == "vector" else nc.scalar.copy)
                    cp(
                        vaug[:, 1024 * t + 128 * h : 1024 * t + 128 * h + 64],
                        f[:, 64 * h : 64 * (h + 1)],
                    )

            def proj_blk(ws, dst, m, blk, ce="vector"):
                # dst[m][:, 512*blk : 512*(blk+1)] = W_m chunk @ x^T T-block
                f = fill_tile()
                for k in range(KC):
                    nc.tensor.matmul(
                        f[:],
                        ws[k][:, 128 * m : 128 * (m + 1)],
                        xTs[k][:, 512 * blk : 512 * (blk + 1)],
                        start=(k == 0),
                        stop=(k == KC - 1),
                    )
                cp = nc.vector.tensor_copy if ce == "vector" else nc.scalar.copy
                cp(dst[m][:, 512 * blk : 512 * (blk + 1)], f[:])

            def out_half(t, n, ce="vector", pool="fill"):
                # out rows 128t..128t+128, cols 512n..512n+512
                ft = fill_tile() if pool == "fill" else sc_tile()
                f = ft if pool == "fill" else ft[:, 0:512]
                for m in range(KF):
                    nc.tensor.matmul(
                        f[:],
                        ctxT[m][:, 128 * t : 128 * (t + 1)],
                        wos[m][:, 512 * n : 512 * (n + 1)],
                        start=(m == 0),
                        stop=(m == KF - 1),
                    )
                o = pot.tile([128, 512], F32, tag="ot", bufs=2, name="ot")
                cp = nc.vector.tensor_copy if ce == "vector" else nc.scalar.copy
                cp(o[:], f[:])
                nc.sync.dma_start(out[128 * t : 128 * (t + 1), 512 * n : 512 * (n + 1)], o[:])

            def attn_group(j, p, fillers, pending):
                """Attention for head pair p (heads 2p, 2p+1), q-block j.
                fillers: callables interleaved evenly into the s-loop.
                pending: previous group's last-AV + normalize closures,
                emitted after this group's first exp (cross-group pipeline
                so the previous AV stall cannot block this scores stream).
                Returns this group's own pending list."""
                pcx = [
                    ps.tile([128, 512], F32, tag="cx", bufs=3, name="pcx")
                    for _ in range(2)
                ]
                nf = len(fillers)
                emitted = 0
                es = [None] * NS

                def av(s):
                    for half in range(2):
                        h = 2 * p + half
                        nc.tensor.matmul(
                            pcx[half][:],
                            vaug[:, 1024 * s + 128 * h : 1024 * s + 128 * (h + 1)],
                            es[s][:, 512 * half : 512 * (half + 1)],
                            start=(s == 0),
                            stop=(s == NS - 1),
                        )

                for s in range(NS):
                    sc = sc_tile()
                    for half in range(2):
                        lo = 64 * half
                        nc.tensor.matmul(
                            sc[:, 512 * half : 512 * (half + 1)],
                            kT[p][lo : lo + 64, 128 * s : 128 * (s + 1)],
                            qT[p][lo : lo + 64, TQ * j : TQ * (j + 1)],
                            tile_position=(lo, 0),
                        )
                    es[s] = pexp.tile([128, 1024], mmdt, tag="e", bufs=3, name="e")
                    if s in DVE_S:
                        nc.vector.tensor_scalar(
                            es[s][:].bitcast(I16), sc[:], EXP_A, EXP_B, ALU.mult, ALU.add
                        )
                    else:
                        nc.scalar.activation(es[s][:], sc[:], AF.Exp, scale=0.125)
                    if s == 0:
                        for cl in pending:
                            cl()
                    else:
                        av(s - 1)
                    want = (s + 1) * nf // NS
                    ce = "scalar" if s in DVE_S else "vector"
                    while emitted < want:
                        fillers[emitted](ce)
                        emitted += 1

                def normalize():
                    for half in range(2):
                        lo = 64 * half
                        zs = prz.tile([1, 512], F32, tag="zs", bufs=2, name="zs")
                        nc.scalar.copy(zs[:], pcx[half][64:65, :])
                        rcp = prz.tile([1, 512], F32, tag="rcp", bufs=2, name="rcp")
                        nc.vector.reciprocal_approx_fast(rcp[:], zs[:])
                        bcs = prz.tile([64, 512], F32, tag="bcs", bufs=2, name="bcs")
                        nc.gpsimd.partition_broadcast(bcs[:], rcp[:])
                        nc.vector.tensor_mul(
                            ctxT[p][lo : lo + 64, TQ * j : TQ * (j + 1)],
                            pcx[half][0:64, :],
                            bcs[:],
                        )

                return [lambda: av(NS - 1), normalize]

            # ---------------- emission schedule ----------------
            # upfront: minimal gate for attn(0,0): kT[0] blk0, qT[0] blk0,
            # v tiles 0..3 (v t lands JIT at iter t-2 for the rest)
            proj_blk(wks, kT, 0, 0)
            proj_blk(wqs, qT, 0, 0)
            for t in range(4):
                v_tile(t)

            # filler queues per (p, j) group; k0 blk b needed by iter 4b of
            # every p=0 group; v tile t needed at iter t of (0,0); q0 blk b
            # needed by group (0,b).
            fq = {(p, j): [] for p in range(3) for j in range(NJ)}
            f_v = lambda t: (lambda ce, t=t: v_tile(t, ce))
            f_p = lambda ws, dst, m, b: (lambda ce, m=m, b=b: proj_blk(ws, dst, m, b, ce))
            fq[(0, 0)] = [
                f_v(4), f_v(5), f_p(wks, kT, 0, 1), f_v(6), f_v(7),
                f_p(wks, kT, 0, 2), f_v(8), f_v(9), f_v(10),
                f_p(wks, kT, 0, 3), f_v(11), f_v(12), f_v(13), f_v(14),
                f_v(15), f_p(wqs, qT, 0, 1),
            ]
            fq[(0, 1)].append(f_p(wqs, qT, 0, 2))
            fq[(0, 2)].append(f_p(wqs, qT, 0, 3))
            for p in range(3):
                m = p + 1
                chunk = [f_p(wks, kT, m, b) for b in range(NJ)]
                chunk += [f_p(wqs, qT, m, b) for b in range(NJ)]
                slots = [(0, 1), (0, 2), (0, 3)] if p == 0 else [(p, j) for j in range(NJ)]
                for i, f in enumerate(chunk):
                    fq[slots[i % len(slots)]].append(f)

            pending = []
            for p in range(3):
                for j in range(NJ):
                    pending = attn_group(j, p, fq[(p, j)], pending)
            # p=3: out-proj row tiles of block j-1 fill during block j
            for j in range(NJ):
                of = []
                if j >= 1:
                    jj = j - 1
                    of = [
                        (lambda ce, t=t, n=n: out_half(t, n, ce))
                        for t in range(4 * jj, 4 * jj + 4)
                        for n in range(2)
                    ]
                pending = attn_group(j, 3, of, pending)
            for cl in pending:
                cl()
            for i, (t, n) in enumerate([(t, n) for t in range(12, NT) for n in range(2)]):
                out_half(
                    t, n,
                    ce=("vector" if i % 2 == 0 else "scalar"),
                    pool=("fill" if i % 2 == 0 else "sc"),
                )

            if DEBUG:
                nc.sync.dma_start(q_dbg[:, :], qT[0][:])
                nc.sync.dma_start(k_dbg[:, :], kT[0][:])
                nc.sync.dma_start(va_dbg[:, :], vaug[:, 0:2048])
                nc.sync.dma_start(ctx_dbg[:, :], ctxT[0][:])

    nc.compile()
    return nc


_NC_CACHE = {}


def _get_nc(T=2048, V=DIM, F=512):
    key = (T, V, F)
    if key not in _NC_CACHE:
        _NC_CACHE[key] = build_nc(T, V, F)
    return _NC_CACHE[key]


def make_in_maps(x, Wq, Wk, Wv, Wo, np_mmdt):
    B = x.shape[0]
    F = Wq.shape[0] // 2
    in_maps = []
    for c in range(N_CORES):
        b, g = divmod(c, 2)
        rows = slice(g * F, (g + 1) * F)
        in_maps.append(
            {
                "xT": np.ascontiguousarray(x[b].T).astype(np_mmdt),
                "wqT": np.ascontiguousarray(Wq[rows].T).astype(np_mmdt),
                "wkT": np.ascontiguousarray(Wk[rows].T).astype(np_mmdt),
                "wvT": np.ascontiguousarray(Wv[rows].T).astype(np_mmdt),
                "woT": np.ascontiguousarray(Wo[:, rows].T).astype(np_mmdt),
            }
        )
    return in_maps


def kernel(x, Wq, Wk, Wv, Wo, bo, trace=False):
    x = np.asarray(x, np.float32)
    B, T, V = x.shape
    nc = _get_nc(T=T, V=V, F=V // 2)
    np_mmdt = mybir.dt.np(BF16)
    in_maps = make_in_maps(
        x,
        np.asarray(Wq, np.float32),
        np.asarray(Wk, np.float32),
        np.asarray(Wv, np.float32),
        np.asarray(Wo, np.float32),
        np_mmdt,
    )
    res = run_bass_kernel_spmd(nc, in_maps, core_ids=list(range(N_CORES)), trace=trace)
    outs = [r["out"] for r in res.results]
    full = np.empty((B, T, V), np.float32)
    for b in range(B):
        full[b] = outs[2 * b] + outs[2 * b + 1] + np.asarray(bo, np.float32)
    if trace:
        kernel.last_exec_time_ns = res.exec_time_ns
        kernel.last_results = res
    return full


# revision 19
# speedup vs baseline: 1.2483x; 1.0515x over previous
"""Trainium2 Bass kernel for MiniSelfAttention (v2).

Shapes (full problem): x (4, 2048, 1024), Wq/Wk/Wv/Wo (1024, 1024), bo (1024,).
H=16 heads, D=64. out = softmax(q k^T / 8) v  projected by Wo.

Sharding across 8 cores: core c -> batch b = c//2, head-group g = c%2
(8 heads = 512 features per group). Each core computes a partial output
projection (its 512 ctx features x Wo slice); host sums the two partials
per batch and adds the bias.

v2 structure (vs v1 baseline):
  - Single PSUM region, 8 banks: 'sc' [128,1024]f32 x2 (scores, double
    buffered, 4 banks), 'cx' [128,512]f32 x3 (AV accumulators), 'fill'
    [128,512]f32 x1 (projection / v / out-proj tiles).
  - Loop order: head-pair OUTER, q-block (j) INNER. Projections for
    chunk m+1, v tiles, and out-proj row tiles are interleaved into the
    attention s-loops as PE "filler" so the PE stays dense (HAM warm)
    and the serial phase-1 head is minimized.
  - exp split between ScalarE (true exp) and VectorE (Schraudolph
    bit-trick: bf16_bits = round_i16(A*score + B), ~2-3% sawtooth error
    that cancels in softmax) to relieve the ScalarE bottleneck.
  - softmax: Z row (ones column in vaug) -> ScalarE copy to SBUF ->
    DVE reciprocal_approx_fast -> gpsimd partition_broadcast -> one
    fused tensor_mul into ctxT. (reciprocal_approx_fast must NOT read
    PSUM directly - it misreads partition-offset rows of bank-offset
    PSUM tiles.)
"""

import sys

sys.path.insert(0, "/opt/trn_rl_repo")

import numpy as np

import concourse.bacc as bacc
import concourse.mybir as mybir
from concourse import tile
from concourse.bass_utils import run_bass_kernel_spmd

F32 = mybir.dt.float32
BF16 = mybir.dt.bfloat16
I16 = mybir.dt.int16
AF = mybir.ActivationFunctionType
ALU = mybir.AluOpType

DIM = 1024
HEADS = 16
D = 64
N_CORES = 8

LOG2E = 1.4426950408889634
EXP_A = 128.0 * LOG2E / 8.0      # folds the 1/sqrt(D) score scale
EXP_B = 16248.6                  # 127*128 tuned: centers the sawtooth error
DVE_S = (2, 5, 8, 11, 14)        # s-tiles whose exp runs on VectorE

DEBUG = False                    # dbg.py flips this for intermediate dumps


def build_nc(T=2048, V=DIM, F=512, mmdt=BF16):
    H = F // D                   # heads per core (8)
    KC = V // 128                # contraction chunks for projections (8)
    NT = T // 128                # 128-row tiles of T (16)
    TQ = 512                     # q-block width
    NJ = T // TQ                 # q-blocks (4)
    NS = T // 128                # key tiles (16)
    KF = F // 128                # ctx feature chunks (4)

    nc = bacc.Bacc(trn_type="TRN2")
    xT = nc.dram_tensor("xT", [V, T], mmdt, kind="ExternalInput")
    wqT = nc.dram_tensor("wqT", [V, F], mmdt, kind="ExternalInput")
    wkT = nc.dram_tensor("wkT", [V, F], mmdt, kind="ExternalInput")
    wvT = nc.dram_tensor("wvT", [V, F], mmdt, kind="ExternalInput")
    woT = nc.dram_tensor("woT", [F, V], mmdt, kind="ExternalInput")
    out = nc.dram_tensor("out", [T, V], F32, kind="ExternalOutput")
    if DEBUG:
        q_dbg = nc.dram_tensor("q_dbg", [128, T], mmdt, kind="ExternalOutput")
        k_dbg = nc.dram_tensor("k_dbg", [128, T], mmdt, kind="ExternalOutput")
        va_dbg = nc.dram_tensor("va_dbg", [128, 2048], mmdt, kind="ExternalOutput")
        ctx_dbg = nc.dram_tensor("ctx_dbg", [128, T], mmdt, kind="ExternalOutput")

    with tile.TileContext(nc) as tc:
        with (
            tc.tile_pool(name="pers", bufs=1) as pp,
            tc.tile_pool(name="exp", bufs=1) as pexp,
            tc.tile_pool(name="rz", bufs=1) as prz,
            tc.tile_pool(name="ot", bufs=1) as pot,
            tc.tile_pool(name="ps", bufs=1, space="PSUM") as ps,
        ):
            # ---------------- persistent SBUF ----------------
            xTs = [pp.tile([128, T], mmdt, tag=f"xT{k}", name=f"xTs{k}") for k in range(KC)]
            for k in range(KC):
                nc.sync.dma_start(xTs[k][:], xT[128 * k : 128 * (k + 1), :])

            def load_w(dram, nm, cols):
                ws = []
                for k in range(dram.shape[0] // 128):
                    w = pp.tile([128, cols], mmdt, tag=f"{nm}{k}", name=f"{nm}{k}")
                    nc.sync.dma_start(w[:], dram[128 * k : 128 * (k + 1), :])
                    ws.append(w)
                return ws

            wks = load_w(wkT, "wk", F)
            wqs = load_w(wqT, "wq", F)
            wvs = load_w(wvT, "wv", F)
            wos = load_w(woT, "wo", V)

            qT = [pp.tile([128, T], mmdt, tag=f"qT{m}", name=f"qT{m}") for m in range(KF)]
            kT = [pp.tile([128, T], mmdt, tag=f"kT{m}", name=f"kT{m}") for m in range(KF)]
            ctxT = [pp.tile([128, T], mmdt, tag=f"cT{m}", name=f"cT{m}") for m in range(KF)]
            # vaug: per key-tile t, per head h: [v(64) | 1 | 0*63] at cols 1024*t+128*h
            vaug = pp.tile([128, NT * 128 * H], mmdt, tag="vaug", name="vaug")
            nc.vector.memset(vaug[:], 0.0)
            nc.vector.memset(vaug[:, 64::128], 1.0)

            # ---------------- emission helpers ----------------
            def fill_tile():
                return ps.tile([128, 512], F32, tag="fill", bufs=1, name="fl")

            def sc_tile():
                return ps.tile([128, 1024], F32, tag="sc", bufs=2, name="sc")

            def v_tile(t, ce="vector"):
                f = fill_tile()
                for k in range(KC):
                    nc.tensor.matmul(
                        f[:],
                        xTs[k][:, 128 * t : 128 * (t + 1)],
                        wvs[k][:],
                        start=(k == 0),
                        stop=(k == KC - 1),
                    )
                eng = nc.vector if ce == "vector" else nc.scalar
                for h in range(H):
                    cp = (nc.vector.tensor_copy if ce == "vector" else nc.scalar.copy)
                    cp(
                        vaug[:, 1024 * t + 128 * h : 1024 * t + 128 * h + 64],
                        f[:, 64 * h : 64 * (h + 1)],
                    )

            def proj_blk(ws, dst, m, blk, ce="vector"):
                # dst[m][:, 512*blk : 512*(blk+1)] = W_m chunk @ x^T T-block
                f = fill_tile()
                for k in range(KC):
                    nc.tensor.matmul(
                        f[:],
                        ws[k][:, 128 * m : 128 * (m + 1)],
                        xTs[k][:, 512 * blk : 512 * (blk + 1)],
                        start=(k == 0),
                        stop=(k == KC - 1),
                    )
                cp = nc.vector.tensor_copy if ce == "vector" else nc.scalar.copy
                cp(dst[m][:, 512 * blk : 512 * (blk + 1)], f[:])

            def out_half(t, n, ce="vector", pool="fill"):
                # out rows 128t..128t+128, cols 512n..512n+512
                ft = fill_tile() if pool == "fill" else sc_tile()
                f = ft if pool == "fill" else ft[:, 0:512]
                for m in range(KF):
                    nc.tensor.matmul(
                        f[:],
                        ctxT[m][:, 128 * t : 128 * (t + 1)],
                        wos[m][:, 512 * n : 512 * (n + 1)],
                        start=(m == 0),
                        stop=(m == KF - 1),
                    )
                o = pot.tile([128, 512], F32, tag="ot", bufs=2, name="ot")
                cp = nc.vector.tensor_copy if ce == "vector" else nc.scalar.copy
                cp(o[:], f[:])
                nc.sync.dma_start(out[128 * t : 128 * (t + 1), 512 * n : 512 * (n + 1)], o[:])

            def attn_group(j, p, fillers, pending):
                """Attention for head pair p (heads 2p, 2p+1), q-block j.
                fillers: callables interleaved evenly into the s-loop.
                pending: previous group's last-AV + normalize closures,
                emitted after this group's first exp (cross-group pipeline
                so the previous AV stall cannot block this scores stream).
                Returns this group's own pending list."""
                pcx = [
                    ps.tile([128, 512], F32, tag="cx", bufs=3, name="pcx")
                    for _ in range(2)
                ]
                nf = len(fillers)
                emitted = 0
                es = [None] * NS

                def av(s):
                    for half in range(2):
                        h = 2 * p + half
                        nc.tensor.matmul(
                            pcx[half][:],
                            vaug[:, 1024 * s + 128 * h : 1024 * s + 128 * (h + 1)],
                            es[s][:, 512 * half : 512 * (half + 1)],
                            start=(s == 0),
                            stop=(s == NS - 1),
                        )

                for s in range(NS):
                    sc = sc_tile()
                    for half in range(2):
                        lo = 64 * half
                        nc.tensor.matmul(
                            sc[:, 512 * half : 512 * (half + 1)],
                            kT[p][lo : lo + 64, 128 * s : 128 * (s + 1)],
                            qT[p][lo : lo + 64, TQ * j : TQ * (j + 1)],
                            tile_position=(lo, 0),
                        )
                    es[s] = pexp.tile([128, 1024], mmdt, tag="e", bufs=3, name="e")
                    if s in DVE_S:
                        nc.vector.tensor_scalar(
                            es[s][:].bitcast(I16), sc[:], EXP_A, EXP_B, ALU.mult, ALU.add
                        )
                    else:
                        nc.scalar.activation(es[s][:], sc[:], AF.Exp, scale=0.125)
                    if s == 0:
                        for cl in pending:
                            cl()
                    elif s >= 2:
                        av(s - 2)
                    want = min(nf, (s + 1) * nf // (NS - 2))
                    ce = "scalar" if s in DVE_S else "vector"
                    while emitted < want:
                        fillers[emitted](ce)
                        emitted += 1

                def normalize():
                    for half in range(2):
                        lo = 64 * half
                        zs = prz.tile([1, 512], F32, tag="zs", bufs=2, name="zs")
                        nc.scalar.copy(zs[:], pcx[half][64:65, :])
                        rcp = prz.tile([1, 512], F32, tag="rcp", bufs=2, name="rcp")
                        nc.vector.reciprocal_approx_fast(rcp[:], zs[:])
                        bcs = prz.tile([64, 512], F32, tag="bcs", bufs=2, name="bcs")
                        nc.gpsimd.partition_broadcast(bcs[:], rcp[:])
                        nc.vector.tensor_mul(
                            ctxT[p][lo : lo + 64, TQ * j : TQ * (j + 1)],
                            pcx[half][0:64, :],
                            bcs[:],
                        )

                return [lambda: av(NS - 2), lambda: av(NS - 1), normalize]

            # ---------------- emission schedule ----------------
            # upfront: minimal gate for attn(0,0): kT[0] blk0, qT[0] blk0
            proj_blk(wks, kT, 0, 0)
            proj_blk(wqs, qT, 0, 0)

            # filler queues per (p, j) group; k0 blk b needed by iter 4b of
            # every p=0 group; v tile t needed at iter t of (0,0); q0 blk b
            # needed by group (0,b).
            fq = {(p, j): [] for p in range(3) for j in range(NJ)}
            f_v = lambda t: (lambda ce, t=t: v_tile(t, ce))
            f_p = lambda ws, dst, m, b: (lambda ce, m=m, b=b: proj_blk(ws, dst, m, b, ce))
            fq[(0, 0)] = [
                f_v(0), f_v(1), f_v(2), f_v(3), f_p(wks, kT, 0, 1),
                f_v(4), f_v(5), f_v(6), f_p(wks, kT, 0, 2),
                f_v(7), f_v(8), f_v(9), f_p(wks, kT, 0, 3),
                f_v(10), f_v(11), f_v(12), f_v(13), f_v(14), f_v(15),
                f_p(wqs, qT, 0, 1),
            ]
            fq[(0, 1)].append(f_p(wqs, qT, 0, 2))
            fq[(0, 2)].append(f_p(wqs, qT, 0, 3))
            for p in range(3):
                m = p + 1
                chunk = [f_p(wks, kT, m, b) for b in range(NJ)]
                chunk += [f_p(wqs, qT, m, b) for b in range(NJ)]
                slots = [(0, 1), (0, 2), (0, 3)] if p == 0 else [(p, j) for j in range(NJ)]
                for i, f in enumerate(chunk):
                    fq[slots[i % len(slots)]].append(f)

            pending = []
            for p in range(3):
                for j in range(NJ):
                    pending = attn_group(j, p, fq[(p, j)], pending)
            # p=3: out-proj row tiles of block j-1 fill during block j
            for j in range(NJ):
                of = []
                if j >= 1:
                    jj = j - 1
                    of = [
                        (lambda ce, t=t, n=n: out_half(t, n, ce))
                        for t in range(4 * jj, 4 * jj + 4)
                        for n in range(2)
                    ]
                pending = attn_group(j, 3, of, pending)
            for cl in pending:
                cl()
            for i, (t, n) in enumerate([(t, n) for t in range(12, NT) for n in range(2)]):
                out_half(
                    t, n,
                    ce=("vector" if i % 2 == 0 else "scalar"),
                    pool=("fill" if i % 2 == 0 else "sc"),
                )

            if DEBUG:
                nc.sync.dma_start(q_dbg[:, :], qT[0][:])
                nc.sync.dma_start(k_dbg[:, :], kT[0][:])
                nc.sync.dma_start(va_dbg[:, :], vaug[:, 0:2048])
                nc.sync.dma_start(ctx_dbg[:, :], ctxT[0][:])

    nc.compile()
    return nc


_NC_CACHE = {}


def _get_nc(T=2048, V=DIM, F=512):
    key = (T, V, F)
    if key not in _NC_CACHE:
        _NC_CACHE[key] = build_nc(T, V, F)
    return _NC_CACHE[key]


def make_in_maps(x, Wq, Wk, Wv, Wo, np_mmdt):
    B = x.shape[0]
    F = Wq.shape[0] // 2
    in_maps = []
    for c in range(N_CORES):
        b, g = divmod(c, 2)
        rows = slice(g * F, (g + 1) * F)
        in_maps.append(
            {
                "xT": np.ascontiguousarray(x[b].T).astype(np_mmdt),
                "wqT": np.ascontiguousarray(Wq[rows].T).astype(np_mmdt),
                "wkT": np.ascontiguousarray(Wk[rows].T).astype(np_mmdt),
                "wvT": np.ascontiguousarray(Wv[rows].T).astype(np_mmdt),
                "woT": np.ascontiguousarray(Wo[:, rows].T).astype(np_mmdt),
            }
        )
    return in_maps


def kernel(x, Wq, Wk, Wv, Wo, bo, trace=False):
    x = np.asarray(x, np.float32)
    B, T, V = x.shape
    nc = _get_nc(T=T, V=V, F=V // 2)
    np_mmdt = mybir.dt.np(BF16)
    in_maps = make_in_maps(
        x,
        np.asarray(Wq, np.float32),
        np.asarray(Wk, np.float32),
        np.asarray(Wv, np.float32),
        np.asarray(Wo, np.float32),
        np_mmdt,
    )
    res = run_bass_kernel_spmd(nc, in_maps, core_ids=list(range(N_CORES)), trace=trace)
    outs = [r["out"] for r in res.results]
    full = np.empty((B, T, V), np.float32)
    for b in range(B):
        full[b] = outs[2 * b] + outs[2 * b + 1] + np.asarray(bo, np.float32)
    if trace:
        kernel.last_exec_time_ns = res.exec_time_ns
        kernel.last_results = res
    return full


# revision 20
# speedup vs baseline: 1.3565x; 1.0867x over previous
"""Trainium2 Bass kernel for MiniSelfAttention (v2).

Shapes (full problem): x (4, 2048, 1024), Wq/Wk/Wv/Wo (1024, 1024), bo (1024,).
H=16 heads, D=64. out = softmax(q k^T / 8) v  projected by Wo.

Sharding across 8 cores: core c -> batch b = c//2, head-group g = c%2
(8 heads = 512 features per group). Each core computes a partial output
projection (its 512 ctx features x Wo slice); host sums the two partials
per batch and adds the bias.

v2 structure (vs v1 baseline):
  - Single PSUM region, 8 banks: 'sc' [128,1024]f32 x2 (scores, double
    buffered, 4 banks), 'cx' [128,512]f32 x3 (AV accumulators), 'fill'
    [128,512]f32 x1 (projection / v / out-proj tiles).
  - Loop order: head-pair OUTER, q-block (j) INNER. Projections for
    chunk m+1, v tiles, and out-proj row tiles are interleaved into the
    attention s-loops as PE "filler" so the PE stays dense (HAM warm)
    and the serial phase-1 head is minimized.
  - exp split between ScalarE (true exp) and VectorE (Schraudolph
    bit-trick: bf16_bits = round_i16(A*score + B), ~2-3% sawtooth error
    that cancels in softmax) to relieve the ScalarE bottleneck.
  - softmax: Z row (ones column in vaug) -> ScalarE copy to SBUF ->
    DVE reciprocal_approx_fast -> gpsimd partition_broadcast -> one
    fused tensor_mul into ctxT. (reciprocal_approx_fast must NOT read
    PSUM directly - it misreads partition-offset rows of bank-offset
    PSUM tiles.)
"""

import sys

sys.path.insert(0, "/opt/trn_rl_repo")

import numpy as np

import concourse.bacc as bacc
import concourse.mybir as mybir
from concourse import tile
from concourse.bass_utils import run_bass_kernel_spmd

F32 = mybir.dt.float32
BF16 = mybir.dt.bfloat16
I16 = mybir.dt.int16
AF = mybir.ActivationFunctionType
ALU = mybir.AluOpType

DIM = 1024
HEADS = 16
D = 64
N_CORES = 8

LOG2E = 1.4426950408889634
EXP_A = 128.0 * LOG2E / 8.0      # folds the 1/sqrt(D) score scale
EXP_B = 16248.6                  # 127*128 tuned: centers the sawtooth error
DVE_S = (3, 6, 9, 12, 15)        # s-tiles whose exp runs on VectorE

DEBUG = False                    # dbg.py flips this for intermediate dumps


def build_nc(T=2048, V=DIM, F=512, mmdt=BF16):
    H = F // D                   # heads per core (8)
    KC = V // 128                # contraction chunks for projections (8)
    NT = T // 128                # 128-row tiles of T (16)
    TQ = 512                     # q-block width
    NJ = T // TQ                 # q-blocks (4)
    NS = T // 128                # key tiles (16)
    KF = F // 128                # ctx feature chunks (4)

    nc = bacc.Bacc(trn_type="TRN2")
    xT = nc.dram_tensor("xT", [V, T], mmdt, kind="ExternalInput")
    wqT = nc.dram_tensor("wqT", [V, F], mmdt, kind="ExternalInput")
    wkT = nc.dram_tensor("wkT", [V, F], mmdt, kind="ExternalInput")
    wvT = nc.dram_tensor("wvT", [V, F], mmdt, kind="ExternalInput")
    woT = nc.dram_tensor("woT", [F, V], mmdt, kind="ExternalInput")
    out = nc.dram_tensor("out", [T, V], F32, kind="ExternalOutput")
    if DEBUG:
        q_dbg = nc.dram_tensor("q_dbg", [128, T], mmdt, kind="ExternalOutput")
        k_dbg = nc.dram_tensor("k_dbg", [128, T], mmdt, kind="ExternalOutput")
        va_dbg = nc.dram_tensor("va_dbg", [128, 2048], mmdt, kind="ExternalOutput")
        ctx_dbg = nc.dram_tensor("ctx_dbg", [128, T], mmdt, kind="ExternalOutput")

    with tile.TileContext(nc) as tc:
        with (
            tc.tile_pool(name="pers", bufs=1) as pp,
            tc.tile_pool(name="exp", bufs=1) as pexp,
            tc.tile_pool(name="rz", bufs=1) as prz,
            tc.tile_pool(name="ot", bufs=1) as pot,
            tc.tile_pool(name="ps", bufs=1, space="PSUM") as ps,
        ):
            # ---------------- persistent SBUF ----------------
            xTs = [pp.tile([128, T], mmdt, tag=f"xT{k}", name=f"xTs{k}") for k in range(KC)]
            for k in range(KC):
                nc.sync.dma_start(xTs[k][:], xT[128 * k : 128 * (k + 1), :])

            def load_w(dram, nm, cols):
                ws = []
                for k in range(dram.shape[0] // 128):
                    w = pp.tile([128, cols], mmdt, tag=f"{nm}{k}", name=f"{nm}{k}")
                    nc.sync.dma_start(w[:], dram[128 * k : 128 * (k + 1), :])
                    ws.append(w)
                return ws

            wks = load_w(wkT, "wk", F)
            wvs = load_w(wvT, "wv", F)
            wqs = load_w(wqT, "wq", F)
            wos = load_w(woT, "wo", V)

            qT = [pp.tile([128, T], mmdt, tag=f"qT{m}", name=f"qT{m}") for m in range(KF)]
            kT = [pp.tile([128, T], mmdt, tag=f"kT{m}", name=f"kT{m}") for m in range(KF)]
            ctxT = [pp.tile([128, T], mmdt, tag=f"cT{m}", name=f"cT{m}") for m in range(KF)]
            # vaug: per key-tile t, per head h: [v(64) | 1 | 0*63] at cols 1024*t+128*h
            vaug = pp.tile([128, NT * 128 * H], mmdt, tag="vaug", name="vaug")
            nc.vector.memset(vaug[:], 0.0)
            nc.vector.memset(vaug[:, 64::128], 1.0)

            # ---------------- emission helpers ----------------
            def fill_tile():
                return ps.tile([128, 512], F32, tag="fill", bufs=1, name="fl")

            def sc_tile():
                return ps.tile([128, 1024], F32, tag="sc", bufs=2, name="sc")

            def v_tile(t, ce="vector"):
                # alternate psum pools so the evacuation copy of tile t-1
                # overlaps tile t's matmuls (fill has only one buffer)
                if t % 2 == 0:
                    f = fill_tile()
                else:
                    f = sc_tile()[:, 0:512]
                for k in range(KC):
                    nc.tensor.matmul(
                        f[:],
                        xTs[k][:, 128 * t : 128 * (t + 1)],
                        wvs[k][:],
                        start=(k == 0),
                        stop=(k == KC - 1),
                    )
                eng = nc.vector if ce == "vector" else nc.scalar
                for h in range(H):
                    cp = (nc.vector.tensor_copy if ce == "vector" else nc.scalar.copy)
                    cp(
                        vaug[:, 1024 * t + 128 * h : 1024 * t + 128 * h + 64],
                        f[:, 64 * h : 64 * (h + 1)],
                    )

            def proj_blk(ws, dst, m, blk, ce="vector"):
                # dst[m][:, 512*blk : 512*(blk+1)] = W_m chunk @ x^T T-block
                f = fill_tile()
                for k in range(KC):
                    nc.tensor.matmul(
                        f[:],
                        ws[k][:, 128 * m : 128 * (m + 1)],
                        xTs[k][:, 512 * blk : 512 * (blk + 1)],
                        start=(k == 0),
                        stop=(k == KC - 1),
                    )
                cp = nc.vector.tensor_copy if ce == "vector" else nc.scalar.copy
                cp(dst[m][:, 512 * blk : 512 * (blk + 1)], f[:])

            def out_half(t, n, ce="vector", pool="fill"):
                # out rows 128t..128t+128, cols 512n..512n+512
                ft = fill_tile() if pool == "fill" else sc_tile()
                f = ft if pool == "fill" else ft[:, 0:512]
                for m in range(KF):
                    nc.tensor.matmul(
                        f[:],
                        ctxT[m][:, 128 * t : 128 * (t + 1)],
                        wos[m][:, 512 * n : 512 * (n + 1)],
                        start=(m == 0),
                        stop=(m == KF - 1),
                    )
                o = pot.tile([128, 512], F32, tag="ot", bufs=2, name="ot")
                cp = nc.vector.tensor_copy if ce == "vector" else nc.scalar.copy
                cp(o[:], f[:])
                nc.sync.dma_start(out[128 * t : 128 * (t + 1), 512 * n : 512 * (n + 1)], o[:])

            def attn_group(j, p, fillers, pending):
                """Attention for head pair p (heads 2p, 2p+1), q-block j.
                fillers: callables interleaved evenly into the s-loop.
                pending: previous group's last-AV + normalize closures,
                emitted after this group's first exp (cross-group pipeline
                so the previous AV stall cannot block this scores stream).
                Returns this group's own pending list."""
                pcx = [
                    ps.tile([128, 512], F32, tag="cx", bufs=3, name="pcx")
                    for _ in range(2)
                ]
                nf = len(fillers)
                emitted = 0
                es = [None] * NS

                def av(s):
                    for half in range(2):
                        h = 2 * p + half
                        nc.tensor.matmul(
                            pcx[half][:],
                            vaug[:, 1024 * s + 128 * h : 1024 * s + 128 * (h + 1)],
                            es[s][:, 512 * half : 512 * (half + 1)],
                            start=(s == 0),
                            stop=(s == NS - 1),
                        )

                for s in range(NS):
                    sc = sc_tile()
                    for half in range(2):
                        lo = 64 * half
                        nc.tensor.matmul(
                            sc[:, 512 * half : 512 * (half + 1)],
                            kT[p][lo : lo + 64, 128 * s : 128 * (s + 1)],
                            qT[p][lo : lo + 64, TQ * j : TQ * (j + 1)],
                            tile_position=(lo, 0),
                        )
                    es[s] = pexp.tile([128, 1024], mmdt, tag="e", bufs=3, name="e")
                    if s in DVE_S:
                        nc.vector.tensor_scalar(
                            es[s][:].bitcast(I16), sc[:], EXP_A, EXP_B, ALU.mult, ALU.add
                        )
                    else:
                        nc.scalar.activation(es[s][:], sc[:], AF.Exp, scale=0.125)
                    if s == 0:
                        for cl in pending:
                            cl()
                    elif s >= 2:
                        av(s - 2)
                    want = min(nf, (s + 1) * nf // (NS - 2))
                    ce = "scalar" if s in DVE_S else "vector"
                    while emitted < want:
                        fillers[emitted](ce)
                        emitted += 1

                def normalize():
                    for half in range(2):
                        lo = 64 * half
                        zs = prz.tile([1, 512], F32, tag="zs", bufs=2, name="zs")
                        nc.scalar.copy(zs[:], pcx[half][64:65, :])
                        rcp = prz.tile([1, 512], F32, tag="rcp", bufs=2, name="rcp")
                        nc.vector.reciprocal_approx_fast(rcp[:], zs[:])
                        bcs = prz.tile([64, 512], F32, tag="bcs", bufs=2, name="bcs")
                        nc.gpsimd.partition_broadcast(bcs[:], rcp[:])
                        nc.vector.tensor_mul(
                            ctxT[p][lo : lo + 64, TQ * j : TQ * (j + 1)],
                            pcx[half][0:64, :],
                            bcs[:],
                        )

                return [lambda: av(NS - 2), lambda: av(NS - 1), normalize]

            # ---------------- emission schedule ----------------
            # upfront: minimal gate for attn(0,0): kT[0] blk0, qT[0] blk0
            proj_blk(wks, kT, 0, 0)
            proj_blk(wqs, qT, 0, 0)

            # filler queues per (p, j) group; k0 blk b needed by iter 4b of
            # every p=0 group; v tile t needed at iter t of (0,0); q0 blk b
            # needed by group (0,b).
            fq = {(p, j): [] for p in range(3) for j in range(NJ)}
            f_v = lambda t: (lambda ce, t=t: v_tile(t, ce))
            f_p = lambda ws, dst, m, b: (lambda ce, m=m, b=b: proj_blk(ws, dst, m, b, ce))
            fq[(0, 0)] = [
                f_v(0), f_v(1), f_v(2), f_v(3), f_p(wks, kT, 0, 1),
                f_v(4), f_v(5), f_v(6), f_p(wks, kT, 0, 2),
                f_v(7), f_v(8), f_v(9), f_p(wks, kT, 0, 3),
                f_v(10), f_v(11), f_v(12), f_v(13), f_v(14), f_v(15),
                f_p(wqs, qT, 0, 1),
            ]
            fq[(0, 1)].append(f_p(wqs, qT, 0, 2))
            fq[(0, 2)].append(f_p(wqs, qT, 0, 3))
            for p in range(3):
                m = p + 1
                chunk = [f_p(wks, kT, m, b) for b in range(NJ)]
                chunk += [f_p(wqs, qT, m, b) for b in range(NJ)]
                slots = [(0, 1), (0, 2), (0, 3)] if p == 0 else [(p, j) for j in range(NJ)]
                for i, f in enumerate(chunk):
                    fq[slots[i % len(slots)]].append(f)

            pending = []
            for p in range(3):
                for j in range(NJ):
                    pending = attn_group(j, p, fq[(p, j)], pending)
            # p=3: out-proj row tiles of block j-1 fill during block j
            for j in range(NJ):
                of = []
                if j >= 1:
                    jj = j - 1
                    of = [
                        (lambda ce, t=t, n=n: out_half(t, n, ce))
                        for t in range(4 * jj, 4 * jj + 4)
                        for n in range(2)
                    ]
                pending = attn_group(j, 3, of, pending)
            for cl in pending:
                cl()
            for i, (t, n) in enumerate([(t, n) for t in range(12, NT) for n in range(2)]):
                out_half(
                    t, n,
                    ce=("vector" if i % 2 == 0 else "scalar"),
                    pool=("fill" if i % 2 == 0 else "sc"),
                )

            if DEBUG:
                nc.sync.dma_start(q_dbg[:, :], qT[0][:])
                nc.sync.dma_start(k_dbg[:, :], kT[0][:])
                nc.sync.dma_start(va_dbg[:, :], vaug[:, 0:2048])
                nc.sync.dma_start(ctx_dbg[:, :], ctxT[0][:])

    nc.compile()
    return nc


_NC_CACHE = {}


def _get_nc(T=2048, V=DIM, F=512):
    key = (T, V, F)
    if key not in _NC_CACHE:
        _NC_CACHE[key] = build_nc(T, V, F)
    return _NC_CACHE[key]


def make_in_maps(x, Wq, Wk, Wv, Wo, np_mmdt):
    B = x.shape[0]
    F = Wq.shape[0] // 2
    in_maps = []
    for c in range(N_CORES):
        b, g = divmod(c, 2)
        rows = slice(g * F, (g + 1) * F)
        in_maps.append(
            {
                "xT": np.ascontiguousarray(x[b].T).astype(np_mmdt),
                "wqT": np.ascontiguousarray(Wq[rows].T).astype(np_mmdt),
                "wkT": np.ascontiguousarray(Wk[rows].T).astype(np_mmdt),
                "wvT": np.ascontiguousarray(Wv[rows].T).astype(np_mmdt),
                "woT": np.ascontiguousarray(Wo[:, rows].T).astype(np_mmdt),
            }
        )
    return in_maps


def kernel(x, Wq, Wk, Wv, Wo, bo, trace=False):
    x = np.asarray(x, np.float32)
    B, T, V = x.shape
    nc = _get_nc(T=T, V=V, F=V // 2)
    np_mmdt = mybir.dt.np(BF16)
    in_maps = make_in_maps(
        x,
        np.asarray(Wq, np.float32),
        np.asarray(Wk, np.float32),
        np.asarray(Wv, np.float32),
        np.asarray(Wo, np.float32),
        np_mmdt,
    )
    res = run_bass_kernel_spmd(nc, in_maps, core_ids=list(range(N_CORES)), trace=trace)
    outs = [r["out"] for r in res.results]
    full = np.empty((B, T, V), np.float32)
    for b in range(B):
        full[b] = outs[2 * b] + outs[2 * b + 1] + np.asarray(bo, np.float32)
    if trace:
        kernel.last_exec_time_ns = res.exec_time_ns
        kernel.last_results = res
    return full


# revision 21
# speedup vs baseline: 1.3632x; 1.0050x over previous
"""Trainium2 Bass kernel for MiniSelfAttention (v2).

Shapes (full problem): x (4, 2048, 1024), Wq/Wk/Wv/Wo (1024, 1024), bo (1024,).
H=16 heads, D=64. out = softmax(q k^T / 8) v  projected by Wo.

Sharding across 8 cores: core c -> batch b = c//2, head-group g = c%2
(8 heads = 512 features per group). Each core computes a partial output
projection (its 512 ctx features x Wo slice); host sums the two partials
per batch and adds the bias.

v2 structure (vs v1 baseline):
  - Single PSUM region, 8 banks: 'sc' [128,1024]f32 x2 (scores, double
    buffered, 4 banks), 'cx' [128,512]f32 x3 (AV accumulators), 'fill'
    [128,512]f32 x1 (projection / v / out-proj tiles).
  - Loop order: head-pair OUTER, q-block (j) INNER. Projections for
    chunk m+1, v tiles, and out-proj row tiles are interleaved into the
    attention s-loops as PE "filler" so the PE stays dense (HAM warm)
    and the serial phase-1 head is minimized.
  - exp split between ScalarE (true exp) and VectorE (Schraudolph
    bit-trick: bf16_bits = round_i16(A*score + B), ~2-3% sawtooth error
    that cancels in softmax) to relieve the ScalarE bottleneck.
  - softmax: Z row (ones column in vaug) -> ScalarE copy to SBUF ->
    DVE reciprocal_approx_fast -> gpsimd partition_broadcast -> one
    fused tensor_mul into ctxT. (reciprocal_approx_fast must NOT read
    PSUM directly - it misreads partition-offset rows of bank-offset
    PSUM tiles.)
"""

import sys

sys.path.insert(0, "/opt/trn_rl_repo")

import numpy as np

import concourse.bacc as bacc
import concourse.mybir as mybir
from concourse import tile
from concourse.bass_utils import run_bass_kernel_spmd

F32 = mybir.dt.float32
BF16 = mybir.dt.bfloat16
I16 = mybir.dt.int16
AF = mybir.ActivationFunctionType
ALU = mybir.AluOpType

DIM = 1024
HEADS = 16
D = 64
N_CORES = 8

LOG2E = 1.4426950408889634
EXP_A = 128.0 * LOG2E / 8.0      # folds the 1/sqrt(D) score scale
EXP_B = 16248.6                  # 127*128 tuned: centers the sawtooth error
DVE_S = (1, 3, 6, 9, 12, 15)     # s-tiles whose exp runs on VectorE

DEBUG = False                    # dbg.py flips this for intermediate dumps


def build_nc(T=2048, V=DIM, F=512, mmdt=BF16):
    H = F // D                   # heads per core (8)
    KC = V // 128                # contraction chunks for projections (8)
    NT = T // 128                # 128-row tiles of T (16)
    TQ = 512                     # q-block width
    NJ = T // TQ                 # q-blocks (4)
    NS = T // 128                # key tiles (16)
    KF = F // 128                # ctx feature chunks (4)

    nc = bacc.Bacc(trn_type="TRN2")
    xT = nc.dram_tensor("xT", [V, T], mmdt, kind="ExternalInput")
    wqT = nc.dram_tensor("wqT", [V, F], mmdt, kind="ExternalInput")
    wkT = nc.dram_tensor("wkT", [V, F], mmdt, kind="ExternalInput")
    wvT = nc.dram_tensor("wvT", [V, F], mmdt, kind="ExternalInput")
    woT = nc.dram_tensor("woT", [F, V], mmdt, kind="ExternalInput")
    out = nc.dram_tensor("out", [T, V], F32, kind="ExternalOutput")
    if DEBUG:
        q_dbg = nc.dram_tensor("q_dbg", [128, T], mmdt, kind="ExternalOutput")
        k_dbg = nc.dram_tensor("k_dbg", [128, T], mmdt, kind="ExternalOutput")
        va_dbg = nc.dram_tensor("va_dbg", [128, 2048], mmdt, kind="ExternalOutput")
        ctx_dbg = nc.dram_tensor("ctx_dbg", [128, T], mmdt, kind="ExternalOutput")

    with tile.TileContext(nc) as tc:
        with (
            tc.tile_pool(name="pers", bufs=1) as pp,
            tc.tile_pool(name="exp", bufs=1) as pexp,
            tc.tile_pool(name="rz", bufs=1) as prz,
            tc.tile_pool(name="ot", bufs=1) as pot,
            tc.tile_pool(name="ps", bufs=1, space="PSUM") as ps,
        ):
            # ---------------- persistent SBUF ----------------
            xTs = [pp.tile([128, T], mmdt, tag=f"xT{k}", name=f"xTs{k}") for k in range(KC)]
            for k in range(KC):
                nc.sync.dma_start(xTs[k][:], xT[128 * k : 128 * (k + 1), :])

            def load_w(dram, nm, cols):
                ws = []
                for k in range(dram.shape[0] // 128):
                    w = pp.tile([128, cols], mmdt, tag=f"{nm}{k}", name=f"{nm}{k}")
                    nc.sync.dma_start(w[:], dram[128 * k : 128 * (k + 1), :])
                    ws.append(w)
                return ws

            wks = load_w(wkT, "wk", F)
            wvs = load_w(wvT, "wv", F)
            wqs = load_w(wqT, "wq", F)
            wos = load_w(woT, "wo", V)

            qT = [pp.tile([128, T], mmdt, tag=f"qT{m}", name=f"qT{m}") for m in range(KF)]
            kT = [pp.tile([128, T], mmdt, tag=f"kT{m}", name=f"kT{m}") for m in range(KF)]
            ctxT = [pp.tile([128, T], mmdt, tag=f"cT{m}", name=f"cT{m}") for m in range(KF)]
            # vaug: per key-tile t, per head h: [v(64) | 1 | 0*63] at cols 1024*t+128*h
            vaug = pp.tile([128, NT * 128 * H], mmdt, tag="vaug", name="vaug")
            nc.vector.memset(vaug[:], 0.0)
            nc.vector.memset(vaug[:, 64::128], 1.0)

            # ---------------- emission helpers ----------------
            def fill_tile():
                return ps.tile([128, 512], F32, tag="fill", bufs=1, name="fl")

            def sc_tile():
                return ps.tile([128, 1024], F32, tag="sc", bufs=2, name="sc")

            def v_tile(t, ce="vector"):
                # alternate psum pools so the evacuation copy of tile t-1
                # overlaps tile t's matmuls (fill has only one buffer)
                if t % 2 == 0:
                    f = fill_tile()
                else:
                    f = sc_tile()[:, 0:512]
                for k in range(KC):
                    nc.tensor.matmul(
                        f[:],
                        xTs[k][:, 128 * t : 128 * (t + 1)],
                        wvs[k][:],
                        start=(k == 0),
                        stop=(k == KC - 1),
                    )
                eng = nc.vector if ce == "vector" else nc.scalar
                for h in range(H):
                    cp = (nc.vector.tensor_copy if ce == "vector" else nc.scalar.copy)
                    cp(
                        vaug[:, 1024 * t + 128 * h : 1024 * t + 128 * h + 64],
                        f[:, 64 * h : 64 * (h + 1)],
                    )

            def proj_blk(ws, dst, m, blk, ce="vector"):
                # dst[m][:, 512*blk : 512*(blk+1)] = W_m chunk @ x^T T-block
                f = fill_tile()
                for k in range(KC):
                    nc.tensor.matmul(
                        f[:],
                        ws[k][:, 128 * m : 128 * (m + 1)],
                        xTs[k][:, 512 * blk : 512 * (blk + 1)],
                        start=(k == 0),
                        stop=(k == KC - 1),
                    )
                cp = nc.vector.tensor_copy if ce == "vector" else nc.scalar.copy
                cp(dst[m][:, 512 * blk : 512 * (blk + 1)], f[:])

            def out_half(t, n, ce="vector", pool="fill"):
                # out rows 128t..128t+128, cols 512n..512n+512
                ft = fill_tile() if pool == "fill" else sc_tile()
                f = ft if pool == "fill" else ft[:, 0:512]
                for m in range(KF):
                    nc.tensor.matmul(
                        f[:],
                        ctxT[m][:, 128 * t : 128 * (t + 1)],
                        wos[m][:, 512 * n : 512 * (n + 1)],
                        start=(m == 0),
                        stop=(m == KF - 1),
                    )
                o = pot.tile([128, 512], F32, tag="ot", bufs=2, name="ot")
                cp = nc.vector.tensor_copy if ce == "vector" else nc.scalar.copy
                cp(o[:], f[:])
                nc.sync.dma_start(out[128 * t : 128 * (t + 1), 512 * n : 512 * (n + 1)], o[:])

            def attn_group(j, p, fillers, pending):
                """Attention for head pair p (heads 2p, 2p+1), q-block j.
                fillers: callables interleaved evenly into the s-loop.
                pending: previous group's last-AV + normalize closures,
                emitted after this group's first exp (cross-group pipeline
                so the previous AV stall cannot block this scores stream).
                Returns this group's own pending list."""
                pcx = [
                    ps.tile([128, 512], F32, tag="cx", bufs=3, name="pcx")
                    for _ in range(2)
                ]
                nf = len(fillers)
                emitted = 0
                es = [None] * NS

                def av(s):
                    for half in range(2):
                        h = 2 * p + half
                        nc.tensor.matmul(
                            pcx[half][:],
                            vaug[:, 1024 * s + 128 * h : 1024 * s + 128 * (h + 1)],
                            es[s][:, 512 * half : 512 * (half + 1)],
                            start=(s == 0),
                            stop=(s == NS - 1),
                        )

                for s in range(NS):
                    sc = sc_tile()
                    for half in range(2):
                        lo = 64 * half
                        nc.tensor.matmul(
                            sc[:, 512 * half : 512 * (half + 1)],
                            kT[p][lo : lo + 64, 128 * s : 128 * (s + 1)],
                            qT[p][lo : lo + 64, TQ * j : TQ * (j + 1)],
                            tile_position=(lo, 0),
                        )
                    es[s] = pexp.tile([128, 1024], mmdt, tag="e", bufs=3, name="e")
                    if s in DVE_S:
                        nc.vector.tensor_scalar(
                            es[s][:].bitcast(I16), sc[:], EXP_A, EXP_B, ALU.mult, ALU.add
                        )
                    else:
                        nc.scalar.activation(es[s][:], sc[:], AF.Exp, scale=0.125)
                    if s < len(pending):
                        pending[s]()
                    elif s >= 2:
                        av(s - 2)
                    want = min(nf, (s + 1) * nf // (NS - 2))
                    ce = "scalar" if s in DVE_S else "vector"
                    while emitted < want:
                        fillers[emitted](ce)
                        emitted += 1

                def normalize():
                    for half in range(2):
                        lo = 64 * half
                        zs = prz.tile([1, 512], F32, tag="zs", bufs=2, name="zs")
                        nc.scalar.copy(zs[:], pcx[half][64:65, :])
                        rcp = prz.tile([1, 512], F32, tag="rcp", bufs=2, name="rcp")
                        nc.vector.reciprocal_approx_fast(rcp[:], zs[:])
                        bcs = prz.tile([64, 512], F32, tag="bcs", bufs=2, name="bcs")
                        nc.gpsimd.partition_broadcast(bcs[:], rcp[:])
                        nc.vector.tensor_mul(
                            ctxT[p][lo : lo + 64, TQ * j : TQ * (j + 1)],
                            pcx[half][0:64, :],
                            bcs[:],
                        )

                def av_last_and_norm():
                    av(NS - 1)
                    normalize()

                return [lambda: av(NS - 2), av_last_and_norm]

            # ---------------- emission schedule ----------------
            # upfront: minimal gate for attn(0,0): kT[0] blk0, qT[0] blk0
            proj_blk(wks, kT, 0, 0)
            proj_blk(wqs, qT, 0, 0)

            # filler queues per (p, j) group; k0 blk b needed by iter 4b of
            # every p=0 group; v tile t needed at iter t of (0,0); q0 blk b
            # needed by group (0,b).
            fq = {(p, j): [] for p in range(3) for j in range(NJ)}
            f_v = lambda t: (lambda ce, t=t: v_tile(t, ce))
            f_p = lambda ws, dst, m, b: (lambda ce, m=m, b=b: proj_blk(ws, dst, m, b, ce))
            fq[(0, 0)] = [
                f_v(0), f_v(1), f_v(2), f_v(3), f_p(wks, kT, 0, 1),
                f_v(4), f_v(5), f_v(6), f_p(wks, kT, 0, 2),
                f_v(7), f_v(8), f_v(9), f_p(wks, kT, 0, 3),
                f_v(10), f_v(11), f_v(12), f_v(13), f_v(14), f_v(15),
                f_p(wqs, qT, 0, 1),
            ]
            fq[(0, 1)].append(f_p(wqs, qT, 0, 2))
            fq[(0, 2)].append(f_p(wqs, qT, 0, 3))
            for p in range(3):
                m = p + 1
                chunk = [f_p(wks, kT, m, b) for b in range(NJ)]
                chunk += [f_p(wqs, qT, m, b) for b in range(NJ)]
                slots = [(0, 1), (0, 2), (0, 3)] if p == 0 else [(p, j) for j in range(NJ)]
                for i, f in enumerate(chunk):
                    fq[slots[i % len(slots)]].append(f)

            pending = []
            for p in range(3):
                for j in range(NJ):
                    pending = attn_group(j, p, fq[(p, j)], pending)
            # p=3: out-proj row tiles of block j-1 fill during block j
            for j in range(NJ):
                of = []
                if j >= 1:
                    jj = j - 1
                    of = [
                        (lambda ce, t=t, n=n: out_half(t, n, ce))
                        for t in range(4 * jj, 4 * jj + 4)
                        for n in range(2)
                    ]
                pending = attn_group(j, 3, of, pending)
            for cl in pending:
                cl()
            for i, (t, n) in enumerate([(t, n) for t in range(12, NT) for n in range(2)]):
                out_half(
                    t, n,
                    ce=("vector" if i % 2 == 0 else "scalar"),
                    pool=("fill" if i % 2 == 0 else "sc"),
                )

            if DEBUG:
                nc.sync.dma_start(q_dbg[:, :], qT[0][:])
                nc.sync.dma_start(k_dbg[:, :], kT[0][:])
                nc.sync.dma_start(va_dbg[:, :], vaug[:, 0:2048])
                nc.sync.dma_start(ctx_dbg[:, :], ctxT[0][:])

    nc.compile()
    return nc


_NC_CACHE = {}


def _get_nc(T=2048, V=DIM, F=512):
    key = (T, V, F)
    if key not in _NC_CACHE:
        _NC_CACHE[key] = build_nc(T, V, F)
    return _NC_CACHE[key]


def make_in_maps(x, Wq, Wk, Wv, Wo, np_mmdt):
    B = x.shape[0]
    F = Wq.shape[0] // 2
    in_maps = []
    for c in range(N_CORES):
        b, g = divmod(c, 2)
        rows = slice(g * F, (g + 1) * F)
        in_maps.append(
            {
                "xT": np.ascontiguousarray(x[b].T).astype(np_mmdt),
                "wqT": np.ascontiguousarray(Wq[rows].T).astype(np_mmdt),
                "wkT": np.ascontiguousarray(Wk[rows].T).astype(np_mmdt),
                "wvT": np.ascontiguousarray(Wv[rows].T).astype(np_mmdt),
                "woT": np.ascontiguousarray(Wo[:, rows].T).astype(np_mmdt),
            }
        )
    return in_maps


def kernel(x, Wq, Wk, Wv, Wo, bo, trace=False):
    x = np.asarray(x, np.float32)
    B, T, V = x.shape
    nc = _get_nc(T=T, V=V, F=V // 2)
    np_mmdt = mybir.dt.np(BF16)
    in_maps = make_in_maps(
        x,
        np.asarray(Wq, np.float32),
        np.asarray(Wk, np.float32),
        np.asarray(Wv, np.float32),
        np.asarray(Wo, np.float32),
        np_mmdt,
    )
    res = run_bass_kernel_spmd(nc, in_maps, core_ids=list(range(N_CORES)), trace=trace)
    outs = [r["out"] for r in res.results]
    full = np.empty((B, T, V), np.float32)
    for b in range(B):
        full[b] = outs[2 * b] + outs[2 * b + 1] + np.asarray(bo, np.float32)
    if trace:
        kernel.last_exec_time_ns = res.exec_time_ns
        kernel.last_results = res
    return full


# revision 22
# speedup vs baseline: 1.3702x; 1.0051x over previous
"""Trainium2 Bass kernel for MiniSelfAttention (v2).

Shapes (full problem): x (4, 2048, 1024), Wq/Wk/Wv/Wo (1024, 1024), bo (1024,).
H=16 heads, D=64. out = softmax(q k^T / 8) v  projected by Wo.

Sharding across 8 cores: core c -> batch b = c//2, head-group g = c%2
(8 heads = 512 features per group). Each core computes a partial output
projection (its 512 ctx features x Wo slice); host sums the two partials
per batch and adds the bias.

v2 structure (vs v1 baseline):
  - Single PSUM region, 8 banks: 'sc' [128,1024]f32 x2 (scores, double
    buffered, 4 banks), 'cx' [128,512]f32 x3 (AV accumulators), 'fill'
    [128,512]f32 x1 (projection / v / out-proj tiles).
  - Loop order: head-pair OUTER, q-block (j) INNER. Projections for
    chunk m+1, v tiles, and out-proj row tiles are interleaved into the
    attention s-loops as PE "filler" so the PE stays dense (HAM warm)
    and the serial phase-1 head is minimized.
  - exp split between ScalarE (true exp) and VectorE (Schraudolph
    bit-trick: bf16_bits = round_i16(A*score + B), ~2-3% sawtooth error
    that cancels in softmax) to relieve the ScalarE bottleneck.
  - softmax: Z row (ones column in vaug) -> ScalarE copy to SBUF ->
    DVE reciprocal_approx_fast -> gpsimd partition_broadcast -> one
    fused tensor_mul into ctxT. (reciprocal_approx_fast must NOT read
    PSUM directly - it misreads partition-offset rows of bank-offset
    PSUM tiles.)
"""

import sys

sys.path.insert(0, "/opt/trn_rl_repo")

import numpy as np

import concourse.bacc as bacc
import concourse.mybir as mybir
from concourse import tile
from concourse.bass_utils import run_bass_kernel_spmd

F32 = mybir.dt.float32
BF16 = mybir.dt.bfloat16
I16 = mybir.dt.int16
AF = mybir.ActivationFunctionType
ALU = mybir.AluOpType

DIM = 1024
HEADS = 16
D = 64
N_CORES = 8

LOG2E = 1.4426950408889634
EXP_A = 128.0 * LOG2E / 8.0      # folds the 1/sqrt(D) score scale
EXP_B = 16248.6                  # 127*128 tuned: centers the sawtooth error
DVE_S = (1, 3, 6, 9, 12, 15)     # s-tiles whose exp runs on VectorE

DEBUG = False                    # dbg.py flips this for intermediate dumps


def build_nc(T=2048, V=DIM, F=512, mmdt=BF16):
    H = F // D                   # heads per core (8)
    KC = V // 128                # contraction chunks for projections (8)
    NT = T // 128                # 128-row tiles of T (16)
    TQ = 512                     # q-block width
    NJ = T // TQ                 # q-blocks (4)
    NS = T // 128                # key tiles (16)
    KF = F // 128                # ctx feature chunks (4)

    nc = bacc.Bacc(trn_type="TRN2")
    xT = nc.dram_tensor("xT", [V, T], mmdt, kind="ExternalInput")
    wqT = nc.dram_tensor("wqT", [V, F], mmdt, kind="ExternalInput")
    wkT = nc.dram_tensor("wkT", [V, F], mmdt, kind="ExternalInput")
    wvT = nc.dram_tensor("wvT", [V, F], mmdt, kind="ExternalInput")
    woT = nc.dram_tensor("woT", [F, V], mmdt, kind="ExternalInput")
    out = nc.dram_tensor("out", [T, V], F32, kind="ExternalOutput")
    if DEBUG:
        q_dbg = nc.dram_tensor("q_dbg", [128, T], mmdt, kind="ExternalOutput")
        k_dbg = nc.dram_tensor("k_dbg", [128, T], mmdt, kind="ExternalOutput")
        va_dbg = nc.dram_tensor("va_dbg", [128, 2048], mmdt, kind="ExternalOutput")
        ctx_dbg = nc.dram_tensor("ctx_dbg", [128, T], mmdt, kind="ExternalOutput")

    with tile.TileContext(nc) as tc:
        with (
            tc.tile_pool(name="pers", bufs=1) as pp,
            tc.tile_pool(name="exp", bufs=1) as pexp,
            tc.tile_pool(name="rz", bufs=1) as prz,
            tc.tile_pool(name="ot", bufs=1) as pot,
            tc.tile_pool(name="ps", bufs=1, space="PSUM") as ps,
        ):
            # ---------------- persistent SBUF ----------------
            xTs = [pp.tile([128, T], mmdt, tag=f"xT{k}", name=f"xTs{k}") for k in range(KC)]
            for k in range(KC):
                nc.sync.dma_start(xTs[k][:], xT[128 * k : 128 * (k + 1), :])

            def load_w(dram, nm, cols):
                ws = []
                for k in range(dram.shape[0] // 128):
                    w = pp.tile([128, cols], mmdt, tag=f"{nm}{k}", name=f"{nm}{k}")
                    nc.sync.dma_start(w[:], dram[128 * k : 128 * (k + 1), :])
                    ws.append(w)
                return ws

            wks = load_w(wkT, "wk", F)
            wvs = load_w(wvT, "wv", F)
            wqs = load_w(wqT, "wq", F)
            wos = load_w(woT, "wo", V)

            qT = [pp.tile([128, T], mmdt, tag=f"qT{m}", name=f"qT{m}") for m in range(KF)]
            kT = [pp.tile([128, T], mmdt, tag=f"kT{m}", name=f"kT{m}") for m in range(KF)]
            ctxT = [pp.tile([128, T], mmdt, tag=f"cT{m}", name=f"cT{m}") for m in range(KF)]
            # vaug: per key-tile t, per head h: [v(64) | 1 | 0*63] at cols 1024*t+128*h
            vaug = pp.tile([128, NT * 128 * H], mmdt, tag="vaug", name="vaug")
            nc.vector.memset(vaug[:], 0.0)
            nc.vector.memset(vaug[:, 64::128], 1.0)

            # ---------------- emission helpers ----------------
            def fill_tile():
                return ps.tile([128, 512], F32, tag="fill", bufs=1, name="fl")

            def sc_tile():
                return ps.tile([128, 1024], F32, tag="sc", bufs=2, name="sc")

            def v_tile(t, ce="vector"):
                # alternate psum pools so the evacuation copy of tile t-1
                # overlaps tile t's matmuls (fill has only one buffer)
                if t % 2 == 0:
                    f = fill_tile()
                else:
                    f = sc_tile()[:, 0:512]
                for k in range(KC):
                    nc.tensor.matmul(
                        f[:],
                        xTs[k][:, 128 * t : 128 * (t + 1)],
                        wvs[k][:],
                        start=(k == 0),
                        stop=(k == KC - 1),
                    )
                eng = nc.vector if ce == "vector" else nc.scalar
                for h in range(H):
                    cp = (nc.vector.tensor_copy if ce == "vector" else nc.scalar.copy)
                    cp(
                        vaug[:, 1024 * t + 128 * h : 1024 * t + 128 * h + 64],
                        f[:, 64 * h : 64 * (h + 1)],
                    )

            def proj_blk(ws, dst, m, blk, ce="vector", pool="fill"):
                # dst[m][:, 512*blk : 512*(blk+1)] = W_m chunk @ x^T T-block
                f = fill_tile() if pool == "fill" else sc_tile()[:, 0:512]
                for k in range(KC):
                    nc.tensor.matmul(
                        f[:],
                        ws[k][:, 128 * m : 128 * (m + 1)],
                        xTs[k][:, 512 * blk : 512 * (blk + 1)],
                        start=(k == 0),
                        stop=(k == KC - 1),
                    )
                cp = nc.vector.tensor_copy if ce == "vector" else nc.scalar.copy
                cp(dst[m][:, 512 * blk : 512 * (blk + 1)], f[:])

            def out_half(t, n, ce="vector", pool="fill"):
                # out rows 128t..128t+128, cols 512n..512n+512
                ft = fill_tile() if pool == "fill" else sc_tile()
                f = ft if pool == "fill" else ft[:, 0:512]
                for m in range(KF):
                    nc.tensor.matmul(
                        f[:],
                        ctxT[m][:, 128 * t : 128 * (t + 1)],
                        wos[m][:, 512 * n : 512 * (n + 1)],
                        start=(m == 0),
                        stop=(m == KF - 1),
                    )
                o = pot.tile([128, 512], F32, tag="ot", bufs=2, name="ot")
                cp = nc.vector.tensor_copy if ce == "vector" else nc.scalar.copy
                cp(o[:], f[:])
                nc.sync.dma_start(out[128 * t : 128 * (t + 1), 512 * n : 512 * (n + 1)], o[:])

            def attn_group(j, p, fillers, pending):
                """Attention for head pair p (heads 2p, 2p+1), q-block j.
                fillers: callables interleaved evenly into the s-loop.
                pending: previous group's last-AV + normalize closures,
                emitted after this group's first exp (cross-group pipeline
                so the previous AV stall cannot block this scores stream).
                Returns this group's own pending list."""
                pcx = [
                    ps.tile([128, 512], F32, tag="cx", bufs=3, name="pcx")
                    for _ in range(2)
                ]
                nf = len(fillers)
                emitted = 0
                es = [None] * NS

                def av(s):
                    for half in range(2):
                        h = 2 * p + half
                        nc.tensor.matmul(
                            pcx[half][:],
                            vaug[:, 1024 * s + 128 * h : 1024 * s + 128 * (h + 1)],
                            es[s][:, 512 * half : 512 * (half + 1)],
                            start=(s == 0),
                            stop=(s == NS - 1),
                        )

                for s in range(NS):
                    sc = sc_tile()
                    for half in range(2):
                        lo = 64 * half
                        nc.tensor.matmul(
                            sc[:, 512 * half : 512 * (half + 1)],
                            kT[p][lo : lo + 64, 128 * s : 128 * (s + 1)],
                            qT[p][lo : lo + 64, TQ * j : TQ * (j + 1)],
                            tile_position=(lo, 0),
                        )
                    es[s] = pexp.tile([128, 1024], mmdt, tag="e", bufs=3, name="e")
                    if s in DVE_S:
                        nc.vector.tensor_scalar(
                            es[s][:].bitcast(I16), sc[:], EXP_A, EXP_B, ALU.mult, ALU.add
                        )
                    else:
                        nc.scalar.activation(es[s][:], sc[:], AF.Exp, scale=0.125)
                    if s < len(pending):
                        pending[s]()
                    elif s >= 2:
                        av(s - 2)
                    want = min(nf, (s + 1) * nf // (NS - 2))
                    ce = "scalar" if s in DVE_S else "vector"
                    while emitted < want:
                        fillers[emitted](ce)
                        emitted += 1

                def normalize():
                    for half in range(2):
                        lo = 64 * half
                        zs = prz.tile([1, 512], F32, tag="zs", bufs=2, name="zs")
                        nc.scalar.copy(zs[:], pcx[half][64:65, :])
                        rcp = prz.tile([1, 512], F32, tag="rcp", bufs=2, name="rcp")
                        nc.vector.reciprocal_approx_fast(rcp[:], zs[:])
                        bcs = prz.tile([64, 512], F32, tag="bcs", bufs=2, name="bcs")
                        nc.gpsimd.partition_broadcast(bcs[:], rcp[:])
                        nc.vector.tensor_mul(
                            ctxT[p][lo : lo + 64, TQ * j : TQ * (j + 1)],
                            pcx[half][0:64, :],
                            bcs[:],
                        )

                def av_last_and_norm():
                    av(NS - 1)
                    normalize()

                return [lambda: av(NS - 2), av_last_and_norm]

            # ---------------- emission schedule ----------------
            # upfront: minimal gate for attn(0,0): kT[0] blk0, qT[0] blk0
            # (parallel psum chains so the two groups pipeline)
            proj_blk(wks, kT, 0, 0)
            proj_blk(wqs, qT, 0, 0, pool="sc")

            # filler queues per (p, j) group; k0 blk b needed by iter 4b of
            # every p=0 group; v tile t needed at iter t of (0,0); q0 blk b
            # needed by group (0,b).
            fq = {(p, j): [] for p in range(3) for j in range(NJ)}
            f_v = lambda t: (lambda ce, t=t: v_tile(t, ce))
            f_p = lambda ws, dst, m, b: (lambda ce, m=m, b=b: proj_blk(ws, dst, m, b, ce))
            fq[(0, 0)] = [
                f_v(0), f_v(1), f_v(2), f_v(3), f_p(wks, kT, 0, 1),
                f_v(4), f_v(5), f_v(6), f_p(wks, kT, 0, 2),
                f_v(7), f_v(8), f_v(9), f_p(wks, kT, 0, 3),
                f_v(10), f_v(11), f_v(12), f_v(13), f_v(14), f_v(15),
                f_p(wqs, qT, 0, 1),
            ]
            fq[(0, 1)].append(f_p(wqs, qT, 0, 2))
            fq[(0, 2)].append(f_p(wqs, qT, 0, 3))
            for p in range(3):
                m = p + 1
                chunk = [f_p(wks, kT, m, b) for b in range(NJ)]
                chunk += [f_p(wqs, qT, m, b) for b in range(NJ)]
                slots = [(0, 1), (0, 2), (0, 3)] if p == 0 else [(p, j) for j in range(NJ)]
                for i, f in enumerate(chunk):
                    fq[slots[i % len(slots)]].append(f)

            pending = []
            for p in range(3):
                for j in range(NJ):
                    pending = attn_group(j, p, fq[(p, j)], pending)
            # p=3: out-proj row tiles of block j-1 fill during block j
            for j in range(NJ):
                of = []
                if j >= 1:
                    jj = j - 1
                    of = [
                        (lambda ce, t=t, n=n: out_half(t, n, ce))
                        for t in range(4 * jj, 4 * jj + 4)
                        for n in range(2)
                    ]
                pending = attn_group(j, 3, of, pending)
            for cl in pending:
                cl()
            for i, (t, n) in enumerate([(t, n) for t in range(12, NT) for n in range(2)]):
                out_half(
                    t, n,
                    ce=("vector" if i % 2 == 0 else "scalar"),
                    pool=("fill" if i % 2 == 0 else "sc"),
                )

            if DEBUG:
                nc.sync.dma_start(q_dbg[:, :], qT[0][:])
                nc.sync.dma_start(k_dbg[:, :], kT[0][:])
                nc.sync.dma_start(va_dbg[:, :], vaug[:, 0:2048])
                nc.sync.dma_start(ctx_dbg[:, :], ctxT[0][:])

    nc.compile()
    return nc


_NC_CACHE = {}


def _get_nc(T=2048, V=DIM, F=512):
    key = (T, V, F)
    if key not in _NC_CACHE:
        _NC_CACHE[key] = build_nc(T, V, F)
    return _NC_CACHE[key]


def make_in_maps(x, Wq, Wk, Wv, Wo, np_mmdt):
    B = x.shape[0]
    F = Wq.shape[0] // 2
    in_maps = []
    for c in range(N_CORES):
        b, g = divmod(c, 2)
        rows = slice(g * F, (g + 1) * F)
        in_maps.append(
            {
                "xT": np.ascontiguousarray(x[b].T).astype(np_mmdt),
                "wqT": np.ascontiguousarray(Wq[rows].T).astype(np_mmdt),
                "wkT": np.ascontiguousarray(Wk[rows].T).astype(np_mmdt),
                "wvT": np.ascontiguousarray(Wv[rows].T).astype(np_mmdt),
                "woT": np.ascontiguousarray(Wo[:, rows].T).astype(np_mmdt),
            }
        )
    return in_maps


def kernel(x, Wq, Wk, Wv, Wo, bo, trace=False):
    x = np.asarray(x, np.float32)
    B, T, V = x.shape
    nc = _get_nc(T=T, V=V, F=V // 2)
    np_mmdt = mybir.dt.np(BF16)
    in_maps = make_in_maps(
        x,
        np.asarray(Wq, np.float32),
        np.asarray(Wk, np.float32),
        np.asarray(Wv, np.float32),
        np.asarray(Wo, np.float32),
        np_mmdt,
    )
    res = run_bass_kernel_spmd(nc, in_maps, core_ids=list(range(N_CORES)), trace=trace)
    outs = [r["out"] for r in res.results]
    full = np.empty((B, T, V), np.float32)
    for b in range(B):
        full[b] = outs[2 * b] + outs[2 * b + 1] + np.asarray(bo, np.float32)
    if trace:
        kernel.last_exec_time_ns = res.exec_time_ns
        kernel.last_results = res
    return full
